# revision 37
# baseline (speedup 1.0000x reference)
"""Bidirectional Conv-Mamba block on 8 Trainium2 NeuronCores.

Sharding: core c = (b = c//2, dir = c%2). Each core runs the full mamba for
its (sample, direction) on a direction-local (possibly reversed) sequence,
plus the direction's half of the tail (mixer conv channel-half + MLP
ffn-half; the pc-conv groups do not mix directions). The only cross-core
exchange is the post-concat LayerNorm sum/sumsq stats: a [2*L] f32
AllReduce between pair cores, with time alignment handled by per-core
input permutation matrices. Host sums the partial outputs during unshard.
"""

import numpy as np

import concourse.bass as bass
import concourse.mybir as mybir
import concourse.tile as tile
from concourse.bass_utils import run_bass_kernel_spmd

F32 = mybir.dt.float32
BF16 = mybir.dt.bfloat16
AF = mybir.ActivationFunctionType
OP = mybir.AluOpType

B, L, D = 4, 2048, 512
DI, DS, DTR, K4 = 1024, 32, 32, 4
_NO_COLLECTIVE = False
P = 128
CB = D // P          # 4 col-blocks of D
DB = DI // P         # 8 d-blocks of DI
TC = 512             # matmul t-chunk
NTC = L // TC
LP = L // P          # 16


def _split_excess_waits(nc):
    """This toolchain's walrus accepts at most one semaphore wait per
    instruction; hoist extra waits onto NoOp carriers placed just before."""
    for f in nc.m.functions:
        for blk in f.blocks:
            insts = blk.instructions  # live list
            i = 0
            k = 0
            while i < len(insts):
                inst = insts[i]
                si = getattr(inst, "sync_info", None)
                if si is not None and si.on_wait and len(si.on_wait) > 1:
                    waits = list(si.on_wait)
                    for w in waits[:-1]:
                        nop = mybir.InstNoOp(name=f"wc{k}_{inst.name}", ins=[], outs=[])
                        nop.engine = inst.engine
                        nop.sync_info = mybir.SyncInfo(on_wait=[w], on_update=[])
                        insts.insert(i, nop)
                        i += 1
                        k += 1
                    inst.sync_info = mybir.SyncInfo(
                        on_wait=[waits[-1]], on_update=list(si.on_update)
                    )
                i += 1


def _build():
    nc = bass.Bass("TRN2", num_devices=8)

    di = lambda n, s: nc.dram_tensor(n, s, F32, kind="ExternalInput")
    dib = lambda n, s: nc.dram_tensor(n, s, BF16, kind="ExternalInput")

    T = {}
    T["x_seq"] = di("x_seq", [L, D])
    T["w_in_T"] = dib("w_in_T", [D, 2 * DI])
    T["lc_w"] = di("lc_w", [P, CB, 3])
    T["lc_b"] = di("lc_b", [P, CB])
    T["norm_w"] = di("norm_w", [P, CB])
    T["lnc_w"] = di("lnc_w", [P, CB])
    T["lnc_b"] = di("lnc_b", [P, CB])
    T["cv_w"] = di("cv_w", [P, DB, K4])
    T["cv_b"] = di("cv_b", [P, DB])
    T["xp_wT"] = dib("xp_wT", [DI, DTR + 2 * DS])
    T["dtp_wT"] = dib("dtp_wT", [DTR, DI])
    T["dtp_b"] = di("dtp_b", [P, DB])
    T["A_dn"] = di("A_dn", [P, DB, DS])
    T["Dp_dn"] = di("Dp_dn", [P, DB])
    T["w_out_T"] = dib("w_out_T", [DI, D])
    T["lnp_w"] = di("lnp_w", [P, CB])
    T["lnp_b"] = di("lnp_b", [P, CB])
    T["pc_w"] = di("pc_w", [P, 2, 6])   # [g, gh, i*3+k]
    T["pc_b"] = di("pc_b", [P, 2])
    T["w1T"] = dib("w1T", [D, DI])      # ffn half
    T["b1"] = di("b1", [P, DB])
    T["w2T"] = dib("w2T", [DI, D])
    T["perm128"] = di("perm128", [P, P])
    T["perm32"] = di("perm32", [2 * LP, 2 * LP])

    T["out_mlp"] = nc.dram_tensor("out_mlp", [D, L], F32, kind="ExternalOutput")
    T["out_mix"] = nc.dram_tensor("out_mix", [D // 2, L], F32, kind="ExternalOutput")

    T["cc_in"] = nc.dram_tensor("cc_in", [1, 2 * L], F32)
    T["cc_out"] = nc.dram_tensor("cc_out", [1, 2 * L], F32)

    with tile.TileContext(nc) as tc:
        _emit(nc, tc, T)

    _split_excess_waits(nc)
    return nc


def _emit(nc, tc, T):
    from contextlib import ExitStack
    from concourse.masks import make_identity

    TS = 512           # scan time-chunk
    NQ = L // TS       # 4

    with ExitStack() as top:
        consts = top.enter_context(tc.tile_pool(name="consts", bufs=1))
        small = top.enter_context(tc.tile_pool(name="small", bufs=2))
        dram = top.enter_context(tc.tile_pool(name="dram", bufs=2, space="PSUM" if False else "DRAM"))

        def cload(name):
            src = T[name][:]
            t = consts.tile(list(src.shape), src.dtype, tag=f"c_{name}")
            nc.sync.dma_start(t[:], src)
            return t

        lc_w_s = cload("lc_w"); lc_b_s = cload("lc_b")
        norm_w_s = cload("norm_w")
        lnc_w_s = cload("lnc_w"); lnc_b_s = cload("lnc_b")
        cv_w_s = cload("cv_w"); cv_b_s = cload("cv_b")
        dtp_b_s = cload("dtp_b"); A_s = cload("A_dn"); Dp_s = cload("Dp_dn")
        lnp_w_s = cload("lnp_w"); lnp_b_s = cload("lnp_b")
        pc_w_s = cload("pc_w"); pc_b_s = cload("pc_b")
        b1_s = cload("b1")
        perm128_s = cload("perm128"); perm32_s = cload("perm32")

        ident = consts.tile([P, P], F32, tag="ident")
        make_identity(nc, ident[:])
        identb = consts.tile([P, P], BF16, tag="identb")
        nc.vector.tensor_copy(identb[:], ident[:])
        ones_f = consts.tile([P, 1], F32, tag="ones_f")
        nc.gpsimd.memset(ones_f[:], 1.0)
        ones_bf = consts.tile([P, 1], BF16, tag="ones_bf")
        nc.gpsimd.memset(ones_bf[:], 1.0)

        def replicate_rowd(rowd, dst_PL):
            nc.sync.dma_start(
                dst_PL[:], rowd[:].rearrange("o t -> (o t)").partition_broadcast(P))

        def tiled_to_rowd(src_sb):
            rowd = dram.tile([1, L], F32, tag="t2r")
            nc.sync.dma_start(rowd[:].rearrange("o (p f) -> (o p) f", p=P), src_sb[:])
            return rowd

        def rowd_to_tiled(rowd_ap, dst_sb):
            nc.sync.dma_start(dst_sb[:], rowd_ap.rearrange("o (p f) -> (o p) f", p=P))

        def rsqrt_tile(v):
            nc.scalar.sqrt(v[:], v[:])
            nc.vector.reciprocal(v[:], v[:])

        # =============== Phase 0-2: xn, xc, ssm_in ========================
        sA = ExitStack()  # ssm_bf: lives to end of in_proj
        ssm_pool = sA.enter_context(tc.tile_pool(name="ssm_pool", bufs=1))
        ssm_bf = ssm_pool.tile([P, CB, L], BF16, tag="ssm_bf")
        xn_bf_d = dram.tile([P, CB, L], BF16, tag="xn_spill")
        with ExitStack() as ph:
            pool = ph.enter_context(tc.tile_pool(name="p02", bufs=2))
            pool1 = ph.enter_context(tc.tile_pool(name="p02a", bufs=1))
            ppsum = ph.enter_context(tc.tile_pool(name="ps02", bufs=2, space="PSUM"))

            # padded bf16 xn slab: [:, cb, 1:1+L] is xn; edges zero for conv3
            xn_bp = pool1.tile([P, CB, 2 + L], BF16, tag="xn_bp")
            nc.vector.memset(xn_bp[:, :, 0:1], 0.0)
            nc.vector.memset(xn_bp[:, :, 1 + L:2 + L], 0.0)
            ms_row_d = dram.tile([1, L], F32, tag="ms_row_d")
            with ExitStack() as ph2:
                pool2 = ph2.enter_context(tc.tile_pool(name="p02b", bufs=1))
                x_d = pool2.tile([P, CB, L], F32, tag="x_d")
                for tt in range(LP):
                    xrow = pool.tile([P, D], F32, tag="xrow")
                    nc.sync.dma_start(xrow[:], T["x_seq"][tt * P:(tt + 1) * P, :])
                    for cb in range(CB):
                        pt = ppsum.tile([P, P], F32, tag="tr")
                        nc.tensor.transpose(pt[:], xrow[:, cb * P:(cb + 1) * P], ident[:])
                        nc.scalar.copy(x_d[:, cb, tt * P:(tt + 1) * P], pt[:])
                # rmsnorm: sumsq over d
                for tcn in range(NTC):
                    ts_ = slice(tcn * TC, (tcn + 1) * TC)
                    pt = ppsum.tile([1, TC], F32, tag="red")
                    for cb in range(CB):
                        sqt = pool.tile([P, TC], F32, tag="sqt")
                        nc.scalar.square(sqt[:], x_d[:, cb, ts_])
                        nc.tensor.matmul(pt[:], ones_f[:], sqt[:],
                                         start=(cb == 0), stop=(cb == CB - 1))
                    prow = small.tile([1, TC], F32, tag="prow")
                    nc.scalar.copy(prow[:], pt[:])
                    nc.sync.dma_start(ms_row_d[:, ts_], prow[:])
                ms_sb = small.tile([P, LP], F32, tag="ms_sb")
                rowd_to_tiled(ms_row_d[:], ms_sb)
                nc.vector.tensor_scalar(ms_sb[:], ms_sb[:], 1.0 / D, 1.1920929e-07,
                                        OP.mult, OP.add)
                rsqrt_tile(ms_sb)
                rs_rep = pool.tile([P, L], F32, tag="rs_rep")
                replicate_rowd(tiled_to_rowd(ms_sb), rs_rep)
                for cb in range(CB):
                    nc.vector.scalar_tensor_tensor(
                        xn_bp[:, cb, 1:1 + L], x_d[:, cb, :], norm_w_s[:, cb:cb + 1],
                        rs_rep[:], OP.mult, OP.mult)

            # conv3 via PE diag matmuls; xc = conv result + bias (bf16)
            diag_lc = pool1.tile([P, CB, 3, P], BF16, tag="diag_lc")
            for cb in range(CB):
                for k in range(3):
                    nc.vector.tensor_scalar(
                        diag_lc[:, cb, k, :], identb[:], lc_w_s[:, cb, k:k + 1],
                        None, OP.mult)
            xc = pool1.tile([P, CB, L], BF16, tag="xc")
            for cb in range(CB):
                for tcn in range(NTC):
                    pt = ppsum.tile([P, TC], F32, tag="c3psum")
                    for k in range(3):
                        nc.tensor.matmul(
                            pt[:], diag_lc[:, cb, k, :],
                            xn_bp[:, cb, k + tcn * TC:k + (tcn + 1) * TC],
                            start=(k == 0), stop=(k == 2))
                    nc.scalar.activation(xc[:, cb, tcn * TC:(tcn + 1) * TC],
                                         pt[:], AF.Identity,
                                         bias=lc_b_s[:, cb:cb + 1])
            # LN over D
            mu_row_d = dram.tile([1, L], F32, tag="mu_row_d")
            ms2_row_d = dram.tile([1, L], F32, tag="ms2_row_d")
            for tcn in range(NTC):
                ts_ = slice(tcn * TC, (tcn + 1) * TC)
                pt = ppsum.tile([1, TC], F32, tag="red")
                for cb in range(CB):
                    nc.tensor.matmul(pt[:], ones_bf[:], xc[:, cb, ts_],
                                     start=(cb == 0), stop=(cb == CB - 1))
                prow = small.tile([1, TC], F32, tag="prow")
                nc.scalar.copy(prow[:], pt[:])
                nc.sync.dma_start(mu_row_d[:, ts_], prow[:])
                pt2 = ppsum.tile([1, TC], F32, tag="red")
                for cb in range(CB):
                    sqt = pool.tile([P, TC], F32, tag="sqt")
                    nc.scalar.square(sqt[:], xc[:, cb, ts_])
                    nc.tensor.matmul(pt2[:], ones_f[:], sqt[:],
                                     start=(cb == 0), stop=(cb == CB - 1))
                prow2 = small.tile([1, TC], F32, tag="prow")
                nc.scalar.copy(prow2[:], pt2[:])
                nc.sync.dma_start(ms2_row_d[:, ts_], prow2[:])
            mu_sb = small.tile([P, LP], F32, tag="mu_sb")
            v_sb = small.tile([P, LP], F32, tag="v_sb")
            rowd_to_tiled(mu_row_d[:], mu_sb)
            rowd_to_tiled(ms2_row_d[:], v_sb)
            nc.vector.tensor_scalar_mul(mu_sb[:], mu_sb[:], 1.0 / D)
            nc.vector.tensor_scalar_mul(v_sb[:], v_sb[:], 1.0 / D)
            mu2 = small.tile([P, LP], F32, tag="mu2")
            nc.vector.tensor_tensor(mu2[:], mu_sb[:], mu_sb[:], OP.mult)
            nc.vector.tensor_sub(v_sb[:], v_sb[:], mu2[:])
            nc.vector.tensor_scalar_add(v_sb[:], v_sb[:], 1e-5)
            rsqrt_tile(v_sb)
            nc.vector.tensor_tensor(mu2[:], mu_sb[:], v_sb[:], OP.mult)
            mr_rep = pool1.tile([P, L], F32, tag="mr_rep")
            rstd_rep = pool1.tile([P, L], F32, tag="rstd_rep")
            replicate_rowd(tiled_to_rowd(mu2), mr_rep)
            replicate_rowd(tiled_to_rowd(v_sb), rstd_rep)
            for cb in range(CB):
                u = pool.tile([P, L], F32, tag="u_ln")
                nc.vector.tensor_tensor(u[:], xc[:, cb, :], rstd_rep[:], OP.mult)
                nc.vector.tensor_sub(u[:], u[:], mr_rep[:])
                nc.vector.tensor_scalar(u[:], u[:], lnc_w_s[:, cb:cb + 1],
                                        lnc_b_s[:, cb:cb + 1], OP.mult, OP.add)
                nc.scalar.activation(u[:], u[:], AF.Silu)
                nc.vector.tensor_add(u[:], u[:], xn_bp[:, cb, 1:1 + L])
                nc.vector.tensor_copy(ssm_bf[:, cb, :], u[:])
                nc.sync.dma_start(xn_bf_d[:, cb, :], xn_bp[:, cb, 1:1 + L])

        # =============== Phase 3: in_proj =================================
        silz_d = dram.tile([P, DB, L], BF16, tag="silz_spill")
        sB = ExitStack()  # xmpre: lives to end of conv4
        xmp_pool = sB.enter_context(tc.tile_pool(name="xmp_pool", bufs=1, side="right"))
        xmpre = xmp_pool.tile([P, DB, 3 + L], BF16, tag="xmpre")
        with ExitStack() as ph:
            pool = ph.enter_context(tc.tile_pool(name="p3", bufs=2))
            pool1 = ph.enter_context(tc.tile_pool(name="p3a", bufs=1))
            ppsum = ph.enter_context(tc.tile_pool(name="ps3", bufs=2, space="PSUM"))
            w_in_s = pool1.tile([P, CB, 2 * DI], BF16, tag="w_in_s")
            nc.sync.dma_start(
                w_in_s[:], T["w_in_T"][:].rearrange("(cb p) j -> p cb j", p=P))
            nc.vector.memset(xmpre[:, :, 0:3], 0.0)
            for tcn in range(NTC):
                ts_ = slice(tcn * TC, (tcn + 1) * TC)
                for jb in range(2 * DB):
                    pt = ppsum.tile([P, TC], F32, tag="mmj")
                    for cb in range(CB):
                        nc.tensor.matmul(pt[:], w_in_s[:, cb, jb * P:(jb + 1) * P],
                                         ssm_bf[:, cb, ts_],
                                         start=(cb == 0), stop=(cb == CB - 1))
                    if jb < DB:
                        nc.scalar.copy(
                            xmpre[:, jb, 3 + tcn * TC:3 + (tcn + 1) * TC], pt[:])
                    else:
                        sz = pool.tile([P, TC], BF16, tag="sz")
                        nc.scalar.activation(sz[:], pt[:], AF.Silu)
                        nc.sync.dma_start(silz_d[:, jb - DB, ts_], sz[:])
        sA.close()  # free ssm_bf

        # =============== Phase 4: conv4 via PE diag matmuls ===============
        sX = ExitStack()  # xm_bf: lives to end of phase 5
        xm_pool = sX.enter_context(tc.tile_pool(name="xm_pool", bufs=1))
        xm_bf = xm_pool.tile([P, DB, L], BF16, tag="xm_bf")
        with ExitStack() as ph:
            pool1 = ph.enter_context(tc.tile_pool(name="p4a", bufs=1))
            ppsum = ph.enter_context(tc.tile_pool(name="ps4", bufs=4, space="PSUM"))
            # diag stationaries: diag(cv_w[:, db, k]) = identity * w (per-row)
            diag_cv = pool1.tile([P, DB, K4, P], BF16, tag="diag_cv")
            for db in range(DB):
                for k in range(K4):
                    nc.vector.tensor_scalar(
                        diag_cv[:, db, k, :], identb[:], cv_w_s[:, db, k:k + 1],
                        None, OP.mult)
            for db in range(DB):
                for tcn in range(NTC):
                    pt = ppsum.tile([P, TC], F32, tag="cpsum")
                    for k in range(K4):
                        nc.tensor.matmul(
                            pt[:], diag_cv[:, db, k, :],
                            xmpre[:, db, k + tcn * TC:k + (tcn + 1) * TC],
                            start=(k == 0), stop=(k == K4 - 1))
                    nc.scalar.activation(xm_bf[:, db, tcn * TC:(tcn + 1) * TC],
                                         pt[:], AF.Silu, bias=cv_b_s[:, db:db + 1])
        sB.close()  # free xmpre

        # =============== Phase 5: projections =============================
        dt_d = dram.tile([P, DB, L], BF16, tag="dt_spill")
        w_d = dram.tile([P, DB, L], BF16, tag="w_spill")
        u_d = dram.tile([P, DB, L], BF16, tag="u_spill")      # Dp*xm
        B_d = dram.tile([DS, L], BF16, tag="B_d")
        C_d = dram.tile([DS, L], BF16, tag="C_d")
        with ExitStack() as ph:
            pool = ph.enter_context(tc.tile_pool(name="p45", bufs=2))
            pool1 = ph.enter_context(tc.tile_pool(name="p45a", bufs=1))
            ppsum = ph.enter_context(tc.tile_pool(name="ps45", bufs=2, space="PSUM"))

            xp_s = pool1.tile([P, DB, DTR + 2 * DS], BF16, tag="xp_s")
            nc.sync.dma_start(
                xp_s[:], T["xp_wT"][:].rearrange("(db p) j -> p db j", p=P))
            dtp_s = pool1.tile([DTR, DI], BF16, tag="dtp_s")
            nc.sync.dma_start(dtp_s[:], T["dtp_wT"][:])
            dtpre = pool1.tile([DTR, L], BF16, tag="dtpre")
            B_bf = pool1.tile([DS, L], BF16, tag="B_bf")
            C_bf = pool1.tile([DS, L], BF16, tag="C_bf")
            for tcn in range(NTC):
                ts_ = slice(tcn * TC, (tcn + 1) * TC)
                pt = ppsum.tile([DTR + 2 * DS, TC], F32, tag="mmxp")
                for db in range(DB):
                    nc.tensor.matmul(pt[:], xp_s[:, db, :], xm_bf[:, db, ts_],
                                     start=(db == 0), stop=(db == DB - 1))
                nc.scalar.copy(dtpre[:, ts_], pt[0:DTR, :])
                nc.scalar.copy(B_bf[:, ts_], pt[DTR:DTR + DS, :])
                nc.scalar.copy(C_bf[:, ts_], pt[DTR + DS:, :])
            for tcn in range(NTC):
                ts_ = slice(tcn * TC, (tcn + 1) * TC)
                for db in range(DB):
                    pt = ppsum.tile([P, TC], F32, tag="mmdt")
                    nc.tensor.matmul(pt[:], dtp_s[:, db * P:(db + 1) * P],
                                     dtpre[:, ts_], start=True, stop=True)
                    ett = pool.tile([P, TC], F32, tag="ett")
                    nc.scalar.activation(ett[:], pt[:], AF.Exp,
                                         bias=dtp_b_s[:, db:db + 1])
                    dtt = pool.tile([P, TC], BF16, tag="dtt")
                    nc.scalar.activation(dtt[:], ett[:], AF.Ln, bias=1.0)
                    nc.sync.dma_start(dt_d[:, db, ts_], dtt[:])
                    wt = pool.tile([P, TC], BF16, tag="wt")
                    nc.vector.tensor_tensor(wt[:], dtt[:], xm_bf[:, db, ts_], OP.mult)
                    nc.sync.dma_start(w_d[:, db, ts_], wt[:])
            for db in range(DB):
                ut = pool.tile([P, L], BF16, tag="ut")
                nc.vector.tensor_scalar(ut[:], xm_bf[:, db, :],
                                        Dp_s[:, db:db + 1], None, OP.mult)
                nc.sync.dma_start(u_d[:, db, :], ut[:])

            nc.sync.dma_start(B_d[:], B_bf[:])
            nc.sync.dma_start(C_d[:], C_bf[:])
        sX.close()  # free xm_bf

        # =============== Phase 6: full-length selective scan ==============
        # Per (db, n): a = exp(A*dt) [Act], b = w*B_n [DVE/Pool TT],
        # h = scan(a,b) [DVE, full L, no carry], s = h*C_n [DVE/Pool TT],
        # y_psum[db] += I @ s [PE identity matmuls, fp32 accumulation].
        # Evac fuses the silu(z) gate: y = psum * silz [DVE STT].
        NG = 8             # n-group size (psum capacity forces evac-merge)
        NGRP = DS // NG    # 4 groups
        yapool = top.enter_context(tc.tile_pool(name="yapool", bufs=1))
        with ExitStack() as ph:
            repool = ph.enter_context(tc.tile_pool(name="repool", bufs=1))
            dwpool = ph.enter_context(tc.tile_pool(name="dwpool", bufs=2))
            abpool = ph.enter_context(tc.tile_pool(name="abpool", bufs=3))
            hpool = ph.enter_context(tc.tile_pool(name="hpool", bufs=3))
            zpool = ph.enter_context(tc.tile_pool(name="zpool", bufs=2))
            ppsum = ph.enter_context(tc.tile_pool(name="ps6", bufs=2, space="PSUM"))

            y_acc = yapool.tile([P, DB, L], BF16, tag="y_acc")
            for g in range(NGRP):
                ns = slice(g * NG, (g + 1) * NG)
                Bg = repool.tile([P, NG, L], BF16, tag="Bg")
                nc.sync.dma_start(
                    Bg[:],
                    B_d[ns, :].rearrange("n t -> (n t)").partition_broadcast(P)
                    .rearrange("p (n t) -> p n t", n=NG))
                Cg = repool.tile([P, NG, L], BF16, tag="Cg")
                nc.sync.dma_start(
                    Cg[:],
                    C_d[ns, :].rearrange("n t -> (n t)").partition_broadcast(P)
                    .rearrange("p (n t) -> p n t", n=NG))

                for db in range(DB):
                    dt_q = dwpool.tile([P, L], BF16, tag="dt_q")
                    nc.sync.dma_start(dt_q[:], dt_d[:, db, :])
                    w_q = dwpool.tile([P, L], BF16, tag="w_q")
                    nc.sync.dma_start(w_q[:], w_d[:, db, :])

                    pts = [ppsum.tile([P, TC], F32, name=f"yp{c}_{g}_{db}",
                                      tag=f"yp{c}") for c in range(NTC)]
                    if g == 0:
                        u_q = dwpool.tile([P, L], BF16, tag="u_q")
                        nc.sync.dma_start(u_q[:], u_d[:, db, :])
                        for c in range(NTC):
                            nc.tensor.matmul(pts[c][:], identb[:],
                                             u_q[:, c * TC:(c + 1) * TC],
                                             start=True, stop=False)
                    for j in range(NG):
                        n = g * NG + j
                        a_t = abpool.tile([P, L], BF16, tag="a_t")
                        nc.scalar.activation(a_t[:], dt_q[:], AF.Exp,
                                             scale=A_s[:, db, n:n + 1])
                        b_t = abpool.tile([P, L], BF16, tag="b_t")
                        nc.vector.tensor_tensor(b_t[:], Bg[:, j, :], w_q[:], OP.mult)
                        h_t = hpool.tile([P, L], BF16, tag="h_t")
                        nc.vector.tensor_tensor_scan(
                            h_t[:], a_t[:], b_t[:], 0.0, OP.mult, OP.add)
                        s_t = hpool.tile([P, L], BF16, tag="s_t")
                        # s feeds only PE (a latency-tolerant sink): run ~7/8
                        # of them on the otherwise-idle Pool engine.
                        eng_s = nc.vector if (n % 8 == 3) else nc.gpsimd
                        eng_s.tensor_tensor(s_t[:], h_t[:], Cg[:, j, :], OP.mult)
                        for c in range(NTC):
                            nc.tensor.matmul(pts[c][:], identb[:],
                                             s_t[:, c * TC:(c + 1) * TC],
                                             start=(g != 0 and j == 0),
                                             stop=(j == NG - 1))
                    # merge psum group into y_acc
                    if g == 0:
                        for c in range(NTC):
                            nc.scalar.copy(
                                y_acc[:, db, c * TC:(c + 1) * TC], pts[c][:])
                    elif g < NGRP - 1:
                        for c in range(NTC):
                            nc.vector.scalar_tensor_tensor(
                                y_acc[:, db, c * TC:(c + 1) * TC], pts[c][:], 1.0,
                                y_acc[:, db, c * TC:(c + 1) * TC],
                                OP.mult, OP.add)
                    else:
                        sz = zpool.tile([P, L], BF16, tag="szg")
                        nc.sync.dma_start(sz[:], silz_d[:, db, :])
                        z_t = zpool.tile([P, L], BF16, tag="z_t")
                        for c in range(NTC):
                            cs = slice(c * TC, (c + 1) * TC)
                            nc.vector.scalar_tensor_tensor(
                                z_t[:, cs], pts[c][:], 1.0, y_acc[:, db, cs],
                                OP.mult, OP.add)
                            nc.vector.tensor_tensor(
                                y_acc[:, db, cs], z_t[:, cs], sz[:, cs], OP.mult)

        # =============== Phase 7: out_proj + stats + LN ===================
        xs_ln_d = dram.tile([D, L], BF16, tag="xs_ln_d")
        with ExitStack() as ph:
            pool = ph.enter_context(tc.tile_pool(name="p7", bufs=2))
            pool1 = ph.enter_context(tc.tile_pool(name="p7a", bufs=1))
            ppsum = ph.enter_context(tc.tile_pool(name="ps7", bufs=2, space="PSUM"))
            ppsum1 = ph.enter_context(tc.tile_pool(name="ps7p", bufs=1, space="PSUM"))

            wout_s = pool1.tile([P, DB, D], BF16, tag="wout_s")
            nc.sync.dma_start(
                wout_s[:], T["w_out_T"][:].rearrange("(db p) o -> p db o", p=P))
            xs_bf = pool1.tile([P, CB, L], BF16, tag="xs_bf")
            for tcn in range(NTC):
                ts_ = slice(tcn * TC, (tcn + 1) * TC)
                for ob in range(CB):
                    pt = ppsum.tile([P, TC], F32, tag="mmo")
                    for db in range(DB):
                        nc.tensor.matmul(pt[:], wout_s[:, db, ob * P:(ob + 1) * P],
                                         y_acc[:, db, ts_],
                                         start=(db == 0), stop=(db == DB - 1))
                    nc.scalar.copy(xs_bf[:, ob, ts_], pt[:])

            st_both_d = dram.tile([1, 2 * L], F32, tag="st_both_d")
            for tcn in range(NTC):
                ts_ = slice(tcn * TC, (tcn + 1) * TC)
                pt = ppsum.tile([1, TC], F32, tag="red2")
                for cb in range(CB):
                    nc.tensor.matmul(pt[:], ones_bf[:], xs_bf[:, cb, ts_],
                                     start=(cb == 0), stop=(cb == CB - 1))
                prow = small.tile([1, TC], F32, tag="prow")
                nc.scalar.copy(prow[:], pt[:])
                nc.sync.dma_start(st_both_d[:, tcn * TC:(tcn + 1) * TC], prow[:])
                pt2 = ppsum.tile([1, TC], F32, tag="red2")
                for cb in range(CB):
                    sqt = pool.tile([P, TC], BF16, tag="sqt2")
                    nc.scalar.square(sqt[:], xs_bf[:, cb, ts_])
                    nc.tensor.matmul(pt2[:], ones_bf[:], sqt[:],
                                     start=(cb == 0), stop=(cb == CB - 1))
                prow2 = small.tile([1, TC], F32, tag="prow")
                nc.scalar.copy(prow2[:], pt2[:])
                nc.sync.dma_start(st_both_d[:, L + tcn * TC:L + (tcn + 1) * TC],
                                  prow2[:])

            LP2 = 2 * LP

            def permute_both(rowd_in_ap, rowd_out_ap):
                # permute BOTH stat halves of a [1, 2L] row in one pass:
                # tiled form [P, (s f)], partition-permute via perm128, then
                # f-permute within each half via blockdiag perm32.
                s_sb = small.tile([P, LP2], F32, tag="perm_in")
                nc.sync.dma_start(
                    s_sb[:, 0:LP],
                    rowd_in_ap[:, 0:L].rearrange("o (p f) -> (o p) f", p=P))
                nc.sync.dma_start(
                    s_sb[:, LP:LP2],
                    rowd_in_ap[:, L:2 * L].rearrange("o (p f) -> (o p) f", p=P))
                pt = ppsum1.tile([P, LP2], F32, tag="permp")
                nc.tensor.matmul(pt[:], perm128_s[:], s_sb[:], start=True, stop=True)
                u_sb = small.tile([P, LP2], F32, tag="perm_u")
                nc.scalar.copy(u_sb[:], pt[:])
                pt2 = ppsum1.tile([LP2, P], F32, tag="permt")
                nc.tensor.transpose(pt2[:], u_sb[:], ident[:])
                ut = small.tile([LP2, P], F32, tag="perm_ut")
                nc.scalar.copy(ut[:], pt2[:])
                pt3 = ppsum1.tile([LP2, P], F32, tag="permt2")
                nc.tensor.matmul(pt3[:], perm32_s[:], ut[:], start=True, stop=True)
                ut2 = small.tile([LP2, P], F32, tag="perm_ut2")
                nc.scalar.copy(ut2[:], pt3[:])
                pt4 = ppsum1.tile([P, LP2], F32, tag="permp2")
                nc.tensor.transpose(pt4[:], ut2[:], ident[0:LP2, 0:LP2])
                s2_sb = small.tile([P, LP2], F32, tag="perm_out")
                nc.scalar.copy(s2_sb[:], pt4[:])
                nc.sync.dma_start(
                    rowd_out_ap[:, 0:L].rearrange("o (p f) -> (o p) f", p=P),
                    s2_sb[:, 0:LP])
                nc.sync.dma_start(
                    rowd_out_ap[:, L:2 * L].rearrange("o (p f) -> (o p) f", p=P),
                    s2_sb[:, LP:LP2])

            permute_both(st_both_d, T["cc_in"])
            if _NO_COLLECTIVE:
                nc.sync.dma_start(T["cc_out"][:], T["cc_in"][:])
            else:
                nc.gpsimd.collective_compute(
                    "AllReduce", OP.add,
                    replica_groups=[[0, 1], [2, 3], [4, 5], [6, 7]],
                    ins=[T["cc_in"][:]], outs=[T["cc_out"][:]],
                )
            back_both_d = dram.tile([1, 2 * L], F32, tag="back_both_d")
            permute_both(T["cc_out"], back_both_d)

            mu3 = small.tile([P, LP], F32, tag="mu3")
            v3 = small.tile([P, LP], F32, tag="v3")
            nc.sync.dma_start(
                mu3[:], back_both_d[:, 0:L].rearrange("o (p f) -> (o p) f", p=P))
            nc.sync.dma_start(
                v3[:], back_both_d[:, L:2 * L].rearrange("o (p f) -> (o p) f", p=P))
            nc.vector.tensor_scalar_mul(mu3[:], mu3[:], 1.0 / (2 * D))
            nc.vector.tensor_scalar_mul(v3[:], v3[:], 1.0 / (2 * D))
            mu32 = small.tile([P, LP], F32, tag="mu32")
            nc.vector.tensor_tensor(mu32[:], mu3[:], mu3[:], OP.mult)
            nc.vector.tensor_sub(v3[:], v3[:], mu32[:])
            nc.vector.tensor_scalar_add(v3[:], v3[:], 1e-5)
            rsqrt_tile(v3)
            nc.vector.tensor_tensor(mu32[:], mu3[:], v3[:], OP.mult)
            mr2_rep = pool1.tile([P, L], F32, tag="mr2_rep")
            rstd2_rep = pool1.tile([P, L], F32, tag="rstd2_rep")
            replicate_rowd(tiled_to_rowd(mu32), mr2_rep)
            replicate_rowd(tiled_to_rowd(v3), rstd2_rep)

            for cb in range(CB):
                u = pool.tile([P, L], F32, tag="u_ln2")
                nc.vector.tensor_tensor(u[:], xs_bf[:, cb, :], rstd2_rep[:], OP.mult)
                nc.vector.tensor_sub(u[:], u[:], mr2_rep[:])
                ub = pool.tile([P, L], BF16, tag="ub_ln2")
                nc.vector.tensor_scalar(ub[:], u[:], lnp_w_s[:, cb:cb + 1],
                                        lnp_b_s[:, cb:cb + 1], OP.mult, OP.add)
                nc.sync.dma_start(xs_ln_d[cb * P:(cb + 1) * P, :], ub[:])

        # =============== Phase 8: mixer conv half (PE diag matmuls) =======
        with ExitStack() as ph:
            pool = ph.enter_context(tc.tile_pool(name="p8", bufs=2))
            pool1 = ph.enter_context(tc.tile_pool(name="p8a", bufs=1))
            ppsum = ph.enter_context(tc.tile_pool(name="ps8", bufs=4, space="PSUM"))
            # padded E/O slabs: [:, gh, 1:1+L] holds data, zero edges
            EO = pool1.tile([P, 2, 2, 2 + L], BF16, tag="EO")   # [p, eo, gh, t]
            nc.vector.memset(EO[:, :, :, 0:1], 0.0)
            nc.vector.memset(EO[:, :, :, 1 + L:2 + L], 0.0)
            xr = xs_ln_d[:].rearrange("(gh p two) t -> p gh two t", p=P, two=2)
            nc.sync.dma_start(EO[:, 0, :, 1:1 + L], xr[:, :, 0, :])
            nc.sync.dma_start(EO[:, 1, :, 1:1 + L], xr[:, :, 1, :])
            diag_pc = pool1.tile([P, 2, 6, P], BF16, tag="diag_pc")
            for gh in range(2):
                for i in range(6):
                    nc.vector.tensor_scalar(
                        diag_pc[:, gh, i, :], identb[:], pc_w_s[:, gh, i:i + 1],
                        None, OP.mult)
            for gh in range(2):
                for tcn in range(NTC):
                    pt = ppsum.tile([P, TC], F32, tag="mxpsum")
                    for eo in range(2):
                        for k in range(3):
                            nc.tensor.matmul(
                                pt[:], diag_pc[:, gh, eo * 3 + k, :],
                                EO[:, eo, gh, k + tcn * TC:k + (tcn + 1) * TC],
                                start=(eo == 0 and k == 0),
                                stop=(eo == 1 and k == 2))
                    mout = pool.tile([P, TC], F32, tag="mout")
                    nc.scalar.activation(mout[:], pt[:], AF.Silu,
                                         bias=pc_b_s[:, gh:gh + 1])
                    nc.sync.dma_start(
                        T["out_mix"][gh * P:(gh + 1) * P, tcn * TC:(tcn + 1) * TC],
                        mout[:])

        # =============== Phase 9: MLP half ================================
        with ExitStack() as ph:
            pool = ph.enter_context(tc.tile_pool(name="p9", bufs=2))
            pool1 = ph.enter_context(tc.tile_pool(name="p9a", bufs=1))
            ppsum = ph.enter_context(tc.tile_pool(name="ps9", bufs=2, space="PSUM"))
            w1_s = pool1.tile([P, CB, DI], BF16, tag="w1_s")
            nc.sync.dma_start(w1_s[:], T["w1T"][:].rearrange("(cb p) h -> p cb h", p=P))
            w2_s = pool1.tile([P, DB, D], BF16, tag="w2_s")
            nc.sync.dma_start(w2_s[:], T["w2T"][:].rearrange("(db p) o -> p db o", p=P))
            xn_bf = pool1.tile([P, CB, L], BF16, tag="xn_bf")
            nc.sync.dma_start(xn_bf[:], xn_bf_d[:])
            g_bf = pool1.tile([P, DB, L], BF16, tag="g_bf")
            for tcn in range(NTC):
                ts_ = slice(tcn * TC, (tcn + 1) * TC)
                for hb in range(DB):
                    pt = ppsum.tile([P, TC], F32, tag="mm1")
                    for cb in range(CB):
                        nc.tensor.matmul(pt[:], w1_s[:, cb, hb * P:(hb + 1) * P],
                                         xn_bf[:, cb, ts_],
                                         start=(cb == 0), stop=(cb == CB - 1))
                    nc.scalar.activation(g_bf[:, hb, ts_], pt[:], AF.Gelu,
                                         bias=b1_s[:, hb:hb + 1])
                for ob in range(CB):
                    pt = ppsum.tile([P, TC], F32, tag="mm2")
                    for hb in range(DB):
                        nc.tensor.matmul(pt[:], w2_s[:, hb, ob * P:(ob + 1) * P],
                                         g_bf[:, hb, ts_],
                                         start=(hb == 0), stop=(hb == DB - 1))
                    ot = pool.tile([P, TC], F32, tag="oml")
                    nc.scalar.copy(ot[:], pt[:])
                    nc.sync.dma_start(T["out_mlp"][ob * P:(ob + 1) * P, ts_], ot[:])


_NC_CACHE = None


def _get_nc():
    global _NC_CACHE
    if _NC_CACHE is None:
        _NC_CACHE = _build()
    return _NC_CACHE


def _prep_core_inputs(inputs, b, rev):
    import ml_dtypes
    f32 = np.float32
    bf16 = ml_dtypes.bfloat16

    def dpart(v, nb):  # [nb*128, ...] -> [128, nb, ...]
        v = np.asarray(v, dtype=f32)
        return np.ascontiguousarray(
            v.reshape(nb, P, *v.shape[1:]).transpose(1, 0, *range(2, v.ndim + 1)))

    x = inputs["x"][b]
    if rev:
        x = x[::-1]
    lc_w = inputs["lc_w"][:, 0, :]
    if rev:
        lc_w = lc_w[:, ::-1]
    lnp_w = inputs["lnp_w"][rev * D:(rev + 1) * D]
    lnp_b = inputs["lnp_b"][rev * D:(rev + 1) * D]
    pc_w = inputs["pc_w"][rev * (D // 2):(rev + 1) * (D // 2)]
    if rev:
        pc_w = pc_w[:, :, ::-1]
    pc_b = inputs["pc_b"][rev * (D // 2):(rev + 1) * (D // 2)]
    hsl = slice(rev * DI, (rev + 1) * DI)
    w1 = inputs["w1"][hsl]
    b1v = inputs["b1"][hsl]
    w2 = inputs["w2"][:, hsl]
    A = -np.exp(inputs["A_log"].astype(np.float64)).astype(f32)
    eye = np.eye(P, dtype=f32)
    rv = np.ascontiguousarray(np.eye(P, dtype=f32)[::-1])
    e16 = np.eye(LP, dtype=f32)
    r16 = np.ascontiguousarray(e16[::-1])
    # blockdiag: f-reversal applied independently to the two stat halves
    # of the [1, 2L] row (tiled as [P, (s f)] -> transposed [(s f), P])
    e32 = np.eye(2 * LP, dtype=f32)
    r32 = np.zeros((2 * LP, 2 * LP), f32)
    r32[0:LP, 0:LP] = r16
    r32[LP:, LP:] = r16

    return {
        "x_seq": np.ascontiguousarray(x, dtype=f32),
        "w_in_T": np.ascontiguousarray(inputs["in_w"].astype(f32).T.astype(bf16)),
        "lc_w": dpart(lc_w, CB),
        "lc_b": dpart(inputs["lc_b"], CB),
        "norm_w": dpart(inputs["norm_w"], CB),
        "lnc_w": dpart(inputs["lnc_w"], CB),
        "lnc_b": dpart(inputs["lnc_b"], CB),
        "cv_w": dpart(inputs["cv_w"][:, 0, :], DB),
        "cv_b": dpart(inputs["cv_b"], DB),
        "xp_wT": np.ascontiguousarray(inputs["xp_w"].astype(f32).T.astype(bf16)),
        "dtp_wT": np.ascontiguousarray(inputs["dtp_w"].astype(f32).T.astype(bf16)),
        "dtp_b": dpart(inputs["dtp_b"], DB),
        "A_dn": dpart(A, DB),
        "Dp_dn": dpart(inputs["Dp"], DB),
        "w_out_T": np.ascontiguousarray(inputs["out_w"].astype(f32).T.astype(bf16)),
        "lnp_w": dpart(lnp_w, CB),
        "lnp_b": dpart(lnp_b, CB),
        "pc_w": dpart(np.ascontiguousarray(pc_w).reshape(D // 2, 6), 2),
        "pc_b": dpart(pc_b, 2),
        "w1T": np.ascontiguousarray(np.asarray(w1, dtype=f32).T.astype(bf16)),
        "b1": dpart(b1v, DB),
        "w2T": np.ascontiguousarray(np.asarray(w2, dtype=f32).T.astype(bf16)),
        "perm128": rv if rev else eye,
        "perm32": r32 if rev else e32,
    }


LAST_RESULTS = None


def kernel(**inputs):
    global LAST_RESULTS
    inputs = {k: np.asarray(v) for k, v in inputs.items()}
    nc = _get_nc()
    in_maps = [_prep_core_inputs(inputs, c // 2, c % 2) for c in range(8)]
    res = run_bass_kernel_spmd(nc, in_maps, core_ids=list(range(8)))
    LAST_RESULTS = res
    out = np.empty((B, L, D), np.float32)
    b2 = inputs["b2"].astype(np.float32)
    for b in range(B):
        mf = res.results[2 * b]
        mb = res.results[2 * b + 1]
        acc = inputs["x"][b].astype(np.float32) + b2[None, :]
        acc += mf["out_mlp"].T
        acc += mb["out_mlp"][:, ::-1].T
        acc[:, 0:D // 2] += mf["out_mix"].T
        acc[:, D // 2:] += mb["out_mix"][:, ::-1].T
        out[b] = acc
    return out



# revision 59
# speedup vs baseline: 1.0067x; 1.0067x over previous
"""Bidirectional Conv-Mamba block on 8 Trainium2 NeuronCores.

Sharding: core c = (b = c//2, dir = c%2). Each core runs the full mamba for
its (sample, direction) on a direction-local (possibly reversed) sequence,
plus the direction's half of the tail (mixer conv channel-half + MLP
ffn-half; the pc-conv groups do not mix directions). The only cross-core
exchange is the post-concat LayerNorm sum/sumsq stats: a [2*L] f32
AllReduce between pair cores, with time alignment handled by per-core
input permutation matrices. Host sums the partial outputs during unshard.
"""

import numpy as np

import concourse.bass as bass
import concourse.mybir as mybir
import concourse.tile as tile
from concourse.bass_utils import run_bass_kernel_spmd

F32 = mybir.dt.float32
BF16 = mybir.dt.bfloat16
AF = mybir.ActivationFunctionType
OP = mybir.AluOpType

B, L, D = 4, 2048, 512
DI, DS, DTR, K4 = 1024, 32, 32, 4
_NO_COLLECTIVE = False
P = 128
CB = D // P          # 4 col-blocks of D
DB = DI // P         # 8 d-blocks of DI
TC = 512             # matmul t-chunk
NTC = L // TC
LP = L // P          # 16


def _split_excess_waits(nc):
    """This toolchain's walrus accepts at most one semaphore wait per
    instruction; hoist extra waits onto NoOp carriers placed just before."""
    for f in nc.m.functions:
        for blk in f.blocks:
            insts = blk.instructions  # live list
            i = 0
            k = 0
            while i < len(insts):
                inst = insts[i]
                si = getattr(inst, "sync_info", None)
                if si is not None and si.on_wait and len(si.on_wait) > 1:
                    waits = list(si.on_wait)
                    for w in waits[:-1]:
                        nop = mybir.InstNoOp(name=f"wc{k}_{inst.name}", ins=[], outs=[])
                        nop.engine = inst.engine
                        nop.sync_info = mybir.SyncInfo(on_wait=[w], on_update=[])
                        insts.insert(i, nop)
                        i += 1
                        k += 1
                    inst.sync_info = mybir.SyncInfo(
                        on_wait=[waits[-1]], on_update=list(si.on_update)
                    )
                i += 1


def _build():
    nc = bass.Bass("TRN2", num_devices=8)

    di = lambda n, s: nc.dram_tensor(n, s, F32, kind="ExternalInput")
    dib = lambda n, s: nc.dram_tensor(n, s, BF16, kind="ExternalInput")

    T = {}
    T["x_seq"] = di("x_seq", [L, D])
    T["w_in_T"] = dib("w_in_T", [D, 2 * DI])
    T["lc_w"] = di("lc_w", [P, CB, 3])
    T["lc_b"] = di("lc_b", [P, CB])
    T["norm_w"] = di("norm_w", [P, CB])
    T["lnc_w"] = di("lnc_w", [P, CB])
    T["lnc_b"] = di("lnc_b", [P, CB])
    T["cv_w"] = di("cv_w", [P, DB, K4])
    T["cv_b"] = di("cv_b", [P, DB])
    T["xp_wT"] = dib("xp_wT", [DI, DTR + 2 * DS])
    T["dtp_wT"] = dib("dtp_wT", [DTR, DI])
    T["dtp_b"] = di("dtp_b", [P, DB])
    T["A_dn"] = di("A_dn", [P, DB, DS])
    T["Dp_dn"] = di("Dp_dn", [P, DB])
    T["w_out_T"] = dib("w_out_T", [DI, D])
    T["lnp_w"] = di("lnp_w", [P, CB])
    T["lnp_b"] = di("lnp_b", [P, CB])
    T["pc_w"] = di("pc_w", [P, 2, 6])   # [g, gh, i*3+k]
    T["pc_b"] = di("pc_b", [P, 2])
    T["w1T"] = dib("w1T", [D, DI])      # ffn half
    T["b1"] = di("b1", [P, DB])
    T["w2T"] = dib("w2T", [DI, D])
    T["perm128"] = di("perm128", [P, P])
    T["perm32"] = di("perm32", [2 * LP, 2 * LP])

    T["out_mlp"] = nc.dram_tensor("out_mlp", [D, L], F32, kind="ExternalOutput")
    T["out_mix"] = nc.dram_tensor("out_mix", [D // 2, L], F32, kind="ExternalOutput")

    T["cc_in"] = nc.dram_tensor("cc_in", [1, 2 * L], F32)
    T["cc_out"] = nc.dram_tensor("cc_out", [1, 2 * L], F32)

    with tile.TileContext(nc) as tc:
        _emit(nc, tc, T)

    _split_excess_waits(nc)
    return nc


def _emit(nc, tc, T):
    from contextlib import ExitStack
    from concourse.masks import make_identity

    TS = 512           # scan time-chunk
    NQ = L // TS       # 4

    with ExitStack() as top:
        consts = top.enter_context(tc.tile_pool(name="consts", bufs=1))
        small = top.enter_context(tc.tile_pool(name="small", bufs=2))
        dram = top.enter_context(tc.tile_pool(name="dram", bufs=2, space="PSUM" if False else "DRAM"))

        def cload(name):
            src = T[name][:]
            t = consts.tile(list(src.shape), src.dtype, tag=f"c_{name}")
            nc.sync.dma_start(t[:], src)
            return t

        lc_w_s = cload("lc_w"); lc_b_s = cload("lc_b")
        norm_w_s = cload("norm_w")
        lnc_w_s = cload("lnc_w"); lnc_b_s = cload("lnc_b")
        cv_w_s = cload("cv_w"); cv_b_s = cload("cv_b")
        dtp_b_s = cload("dtp_b"); A_s = cload("A_dn"); Dp_s = cload("Dp_dn")
        lnp_w_s = cload("lnp_w"); lnp_b_s = cload("lnp_b")
        pc_w_s = cload("pc_w"); pc_b_s = cload("pc_b")
        b1_s = cload("b1")
        perm128_s = cload("perm128"); perm32_s = cload("perm32")

        ident = consts.tile([P, P], F32, tag="ident")
        make_identity(nc, ident[:])
        identb = consts.tile([P, P], BF16, tag="identb")
        nc.vector.tensor_copy(identb[:], ident[:])
        ones_bf = consts.tile([P, 1], BF16, tag="ones_bf")
        nc.gpsimd.memset(ones_bf[:], 1.0)

        def replicate_rowd(rowd, dst_PL):
            nc.sync.dma_start(
                dst_PL[:], rowd[:].rearrange("o t -> (o t)").partition_broadcast(P))

        def rowd_to_tiled(rowd_ap, dst_sb):
            nc.sync.dma_start(dst_sb[:], rowd_ap.rearrange("o (p f) -> (o p) f", p=P))

        def rsqrt_tile(v):
            nc.scalar.sqrt(v[:], v[:])
            nc.vector.reciprocal(v[:], v[:])

        def replicate_tiled_bf_ap(src_f32_ap, dst_PL_bf, tagp):
            # f32 [P,LP] tiled stat -> bf16 row -> partition-broadcast [P,L]
            b = small.tile([P, LP], BF16, name=f"{tagp}_b", tag=f"{tagp}_b")
            nc.vector.tensor_copy(b[:], src_f32_ap)
            rowd = dram.tile([1, L], BF16, name=f"{tagp}_rd", tag=f"{tagp}_rd")
            nc.sync.dma_start(rowd[:].rearrange("o (p f) -> (o p) f", p=P), b[:])
            nc.sync.dma_start(
                dst_PL_bf[:],
                rowd[:].rearrange("o t -> (o t)").partition_broadcast(P))

        def replicate_tiled_bf(src_f32_sb, dst_PL_bf, tagp):
            replicate_tiled_bf_ap(src_f32_sb[:], dst_PL_bf, tagp)

        # =============== Phase 0-2: xn, xc, ssm_in ========================
        sA = ExitStack()  # ssm_bf: lives to end of in_proj
        ssm_pool = sA.enter_context(tc.tile_pool(name="ssm_pool", bufs=1))
        ssm_bf = ssm_pool.tile([P, CB, L], BF16, tag="ssm_bf")
        xn_bf_d = dram.tile([P, CB, L], BF16, tag="xn_spill")
        with ExitStack() as ph:
            pool = ph.enter_context(tc.tile_pool(name="p02", bufs=2))
            pool1 = ph.enter_context(tc.tile_pool(name="p02a", bufs=1))
            ppsum = ph.enter_context(tc.tile_pool(name="ps02", bufs=2, space="PSUM"))

            # padded bf16 xn slab: [:, cb, 1:1+L] is xn; edges zero for conv3
            xn_bp = pool1.tile([P, CB, 2 + L], BF16, tag="xn_bp")
            nc.vector.memset(xn_bp[:, :, 0:1], 0.0)
            nc.vector.memset(xn_bp[:, :, 1 + L:2 + L], 0.0)
            with ExitStack() as ph2:
                pool2 = ph2.enter_context(tc.tile_pool(name="p02b", bufs=1))
                xrows = pool2.tile([P, LP, D], F32, tag="xrows")
                nc.sync.dma_start(
                    xrows[:], T["x_seq"][:].rearrange("(tt p) d -> p tt d", p=P))
                x_d = pool2.tile([P, CB, L], F32, tag="x_d")
                for tt in range(LP):
                    for cb in range(CB):
                        pt = ppsum.tile([P, P], F32, tag="tr")
                        nc.tensor.transpose(
                            pt[:], xrows[:, tt, cb * P:(cb + 1) * P], ident[:])
                        if cb % 2 == 0:
                            nc.scalar.copy(x_d[:, cb, tt * P:(tt + 1) * P], pt[:])
                        else:
                            nc.vector.tensor_copy(
                                x_d[:, cb, tt * P:(tt + 1) * P], pt[:])
                # rmsnorm: mean(x^2) over d via bn_stats on the row layout;
                # result lands f-major tiled: msq[p, tt] = stat[t=tt*128+p]
                stat6 = pool.tile([P, LP, 6], F32, tag="stat6")
                mv = pool.tile([P, LP, 2], F32, tag="mv")
                for tt in range(LP):
                    nc.vector.bn_stats(stat6[:, tt, :], xrows[:, tt, :])
                    nc.vector.bn_aggr(mv[:, tt, :], stat6[:, tt, :])
                msq = small.tile([P, LP], F32, tag="msq")
                nc.vector.tensor_tensor(msq[:], mv[:, :, 0], mv[:, :, 0], OP.mult)
                nc.vector.tensor_add(msq[:], msq[:], mv[:, :, 1])
                nc.vector.tensor_scalar_add(msq[:], msq[:], 1.1920929e-07)
                rsqrt_tile(msq)
                ms_row_d = dram.tile([1, L], F32, tag="ms_row_d")
                nc.sync.dma_start(
                    ms_row_d[:].rearrange("o (f p) -> (o p) f", p=P), msq[:])
                rs_rep = pool.tile([P, L], F32, tag="rs_rep")
                replicate_rowd(ms_row_d, rs_rep)
                for cb in range(CB):
                    nc.vector.scalar_tensor_tensor(
                        xn_bp[:, cb, 1:1 + L], x_d[:, cb, :], norm_w_s[:, cb:cb + 1],
                        rs_rep[:], OP.mult, OP.mult)

            # conv3 via PE diag matmuls; xc = conv result + bias (bf16)
            diag_lc = pool1.tile([P, CB, 3, P], BF16, tag="diag_lc")
            for cb in range(CB):
                for k in range(3):
                    nc.vector.tensor_scalar(
                        diag_lc[:, cb, k, :], identb[:], lc_w_s[:, cb, k:k + 1],
                        None, OP.mult)
            xc = pool1.tile([P, CB, L], BF16, tag="xc")
            for cb in range(CB):
                for tcn in range(NTC):
                    pt = ppsum.tile([P, TC], F32, tag="c3psum")
                    for k in range(3):
                        nc.tensor.matmul(
                            pt[:], diag_lc[:, cb, k, :],
                            xn_bp[:, cb, k + tcn * TC:k + (tcn + 1) * TC],
                            start=(k == 0), stop=(k == 2))
                    nc.scalar.activation(xc[:, cb, tcn * TC:(tcn + 1) * TC],
                                         pt[:], AF.Identity,
                                         bias=lc_b_s[:, cb:cb + 1])
            # LN over D
            mu_row_d = dram.tile([1, L], F32, tag="mu_row_d")
            ms2_row_d = dram.tile([1, L], F32, tag="ms2_row_d")
            for tcn in range(NTC):
                ts_ = slice(tcn * TC, (tcn + 1) * TC)
                pt = ppsum.tile([1, TC], F32, tag="red")
                for cb in range(CB):
                    nc.tensor.matmul(pt[:], ones_bf[:], xc[:, cb, ts_],
                                     start=(cb == 0), stop=(cb == CB - 1))
                prow = small.tile([1, TC], F32, tag="prow")
                nc.scalar.copy(prow[:], pt[:])
                nc.sync.dma_start(mu_row_d[:, ts_], prow[:])
                pt2 = ppsum.tile([1, TC], F32, tag="red")
                for cb in range(CB):
                    sqt = pool.tile([P, TC], BF16, tag="sqt")
                    nc.vector.tensor_tensor(sqt[:], xc[:, cb, ts_], xc[:, cb, ts_],
                                            OP.mult)
                    nc.tensor.matmul(pt2[:], ones_bf[:], sqt[:],
                                     start=(cb == 0), stop=(cb == CB - 1))
                prow2 = small.tile([1, TC], F32, tag="prow")
                nc.scalar.copy(prow2[:], pt2[:])
                nc.sync.dma_start(ms2_row_d[:, ts_], prow2[:])
            mu_sb = small.tile([P, LP], F32, tag="mu_sb")
            v_sb = small.tile([P, LP], F32, tag="v_sb")
            rowd_to_tiled(mu_row_d[:], mu_sb)
            rowd_to_tiled(ms2_row_d[:], v_sb)
            nc.vector.tensor_scalar_mul(mu_sb[:], mu_sb[:], 1.0 / D)
            nc.vector.tensor_scalar_mul(v_sb[:], v_sb[:], 1.0 / D)
            mu2 = small.tile([P, LP], F32, tag="mu2")
            nc.vector.tensor_tensor(mu2[:], mu_sb[:], mu_sb[:], OP.mult)
            nc.vector.tensor_sub(v_sb[:], v_sb[:], mu2[:])
            nc.vector.tensor_scalar_add(v_sb[:], v_sb[:], 1e-5)
            rsqrt_tile(v_sb)
            nc.vector.tensor_tensor(mu2[:], mu_sb[:], v_sb[:], OP.mult)
            mr_rep = pool1.tile([P, L], BF16, tag="mr_rep")
            rstd_rep = pool1.tile([P, L], BF16, tag="rstd_rep")
            replicate_tiled_bf(mu2, mr_rep, "ln1m")
            replicate_tiled_bf(v_sb, rstd_rep, "ln1v")
            for cb in range(CB):
                u = pool.tile([P, L], BF16, tag="u_ln")
                nc.vector.tensor_tensor(u[:], xc[:, cb, :], rstd_rep[:], OP.mult)
                nc.vector.tensor_sub(u[:], u[:], mr_rep[:])
                nc.vector.tensor_scalar(u[:], u[:], lnc_w_s[:, cb:cb + 1],
                                        lnc_b_s[:, cb:cb + 1], OP.mult, OP.add)
                nc.scalar.activation(u[:], u[:], AF.Silu)
                nc.vector.tensor_tensor(ssm_bf[:, cb, :], u[:],
                                        xn_bp[:, cb, 1:1 + L], OP.add)
                nc.sync.dma_start(xn_bf_d[:, cb, :], xn_bp[:, cb, 1:1 + L])

        # =============== Phase 3: in_proj =================================
        silz_d = dram.tile([P, DB, L], BF16, tag="silz_spill")
        sB = ExitStack()  # xmpre: lives to end of conv4
        xmp_pool = sB.enter_context(tc.tile_pool(name="xmp_pool", bufs=1, side="right"))
        xmpre = xmp_pool.tile([P, DB, 3 + L], BF16, tag="xmpre")
        with ExitStack() as ph:
            pool = ph.enter_context(tc.tile_pool(name="p3", bufs=2))
            pool1 = ph.enter_context(tc.tile_pool(name="p3a", bufs=1))
            ppsum = ph.enter_context(tc.tile_pool(name="ps3", bufs=2, space="PSUM"))
            w_in_s = pool1.tile([P, CB, 2 * DI], BF16, tag="w_in_s")
            nc.sync.dma_start(
                w_in_s[:], T["w_in_T"][:].rearrange("(cb p) j -> p cb j", p=P))
            nc.vector.memset(xmpre[:, :, 0:3], 0.0)
            # jb-outer with the xm blocks first: each xmpre[db] completes
            # early so conv4 pipelines with the rest of in_proj; silz last
            for jb in range(2 * DB):
                for tcn in range(NTC):
                    ts_ = slice(tcn * TC, (tcn + 1) * TC)
                    pt = ppsum.tile([P, TC], F32, tag="mmj")
                    for cb in range(CB):
                        nc.tensor.matmul(pt[:], w_in_s[:, cb, jb * P:(jb + 1) * P],
                                         ssm_bf[:, cb, ts_],
                                         start=(cb == 0), stop=(cb == CB - 1))
                    if jb < DB:
                        nc.scalar.copy(
                            xmpre[:, jb, 3 + tcn * TC:3 + (tcn + 1) * TC], pt[:])
                    else:
                        sz = pool.tile([P, TC], BF16, tag="sz")
                        nc.scalar.activation(sz[:], pt[:], AF.Silu)
                        nc.sync.dma_start(silz_d[:, jb - DB, ts_], sz[:])
        sA.close()  # free ssm_bf

        # =============== Phase 4: conv4 via PE diag matmuls ===============
        sX = ExitStack()  # xm_bf: lives to end of phase 5
        xm_pool = sX.enter_context(tc.tile_pool(name="xm_pool", bufs=1))
        xm_bf = xm_pool.tile([P, DB, L], BF16, tag="xm_bf")
        with ExitStack() as ph:
            pool1 = ph.enter_context(tc.tile_pool(name="p4a", bufs=1))
            ppsum = ph.enter_context(tc.tile_pool(name="ps4", bufs=4, space="PSUM"))
            # diag stationaries: diag(cv_w[:, db, k]) = identity * w (per-row)
            diag_cv = pool1.tile([P, DB, K4, P], BF16, tag="diag_cv")
            for db in range(DB):
                for k in range(K4):
                    nc.vector.tensor_scalar(
                        diag_cv[:, db, k, :], identb[:], cv_w_s[:, db, k:k + 1],
                        None, OP.mult)
            for db in range(DB):
                for tcn in range(NTC):
                    pt = ppsum.tile([P, TC], F32, tag="cpsum")
                    for k in range(K4):
                        nc.tensor.matmul(
                            pt[:], diag_cv[:, db, k, :],
                            xmpre[:, db, k + tcn * TC:k + (tcn + 1) * TC],
                            start=(k == 0), stop=(k == K4 - 1))
                    nc.scalar.activation(xm_bf[:, db, tcn * TC:(tcn + 1) * TC],
                                         pt[:], AF.Silu, bias=cv_b_s[:, db:db + 1])
        sB.close()  # free xmpre

        # =============== Phase 5: projections =============================
        dt_d = dram.tile([P, DB, L], BF16, tag="dt_spill")
        w_d = dram.tile([P, DB, L], BF16, tag="w_spill")
        u_d = dram.tile([P, DB, L], BF16, tag="u_spill")      # Dp*xm
        B_d = dram.tile([DS, L], BF16, tag="B_d")
        C_d = dram.tile([DS, L], BF16, tag="C_d")
        with ExitStack() as ph:
            pool = ph.enter_context(tc.tile_pool(name="p45", bufs=2))
            pool1 = ph.enter_context(tc.tile_pool(name="p45a", bufs=1))
            ppsum = ph.enter_context(tc.tile_pool(name="ps45", bufs=2, space="PSUM"))

            xp_s = pool1.tile([P, DB, DTR + 2 * DS], BF16, tag="xp_s")
            nc.sync.dma_start(
                xp_s[:], T["xp_wT"][:].rearrange("(db p) j -> p db j", p=P))
            dtp_s = pool1.tile([DTR, DI], BF16, tag="dtp_s")
            nc.sync.dma_start(dtp_s[:], T["dtp_wT"][:])
            dtpre = pool1.tile([DTR, L], BF16, tag="dtpre")
            B_bf = pool1.tile([DS, L], BF16, tag="B_bf")
            C_bf = pool1.tile([DS, L], BF16, tag="C_bf")
            for tcn in range(NTC):
                ts_ = slice(tcn * TC, (tcn + 1) * TC)
                pt = ppsum.tile([DTR + 2 * DS, TC], F32, tag="mmxp")
                for db in range(DB):
                    nc.tensor.matmul(pt[:], xp_s[:, db, :], xm_bf[:, db, ts_],
                                     start=(db == 0), stop=(db == DB - 1))
                nc.scalar.copy(dtpre[:, ts_], pt[0:DTR, :])
                nc.scalar.copy(B_bf[:, ts_], pt[DTR:DTR + DS, :])
                nc.scalar.copy(C_bf[:, ts_], pt[DTR + DS:, :])
            nc.sync.dma_start(B_d[:], B_bf[:])
            nc.sync.dma_start(C_d[:], C_bf[:])
            # db-outer so phase 6 (which consumes per-db spills) can start on
            # db=0 while later dbs are still being produced
            for db in range(DB):
                for tcn in range(NTC):
                    ts_ = slice(tcn * TC, (tcn + 1) * TC)
                    pt = ppsum.tile([P, TC], F32, tag="mmdt")
                    nc.tensor.matmul(pt[:], dtp_s[:, db * P:(db + 1) * P],
                                     dtpre[:, ts_], start=True, stop=True)
                    ett = pool.tile([P, TC], F32, tag="ett")
                    nc.scalar.activation(ett[:], pt[:], AF.Exp,
                                         bias=dtp_b_s[:, db:db + 1])
                    dtt = pool.tile([P, TC], BF16, tag="dtt")
                    nc.scalar.activation(dtt[:], ett[:], AF.Ln, bias=1.0)
                    nc.sync.dma_start(dt_d[:, db, ts_], dtt[:])
                    wt = pool.tile([P, TC], BF16, tag="wt")
                    nc.vector.tensor_tensor(wt[:], dtt[:], xm_bf[:, db, ts_], OP.mult)
                    nc.sync.dma_start(w_d[:, db, ts_], wt[:])
                ut = pool.tile([P, L], BF16, tag="ut")
                nc.vector.tensor_scalar(ut[:], xm_bf[:, db, :],
                                        Dp_s[:, db:db + 1], None, OP.mult)
                nc.sync.dma_start(u_d[:, db, :], ut[:])
        sX.close()  # free xm_bf

        # =============== Phase 6: full-length selective scan ==============
        # Per (db, n): a = exp(A*dt) [Act], b = w*B_n [DVE/Pool TT],
        # h = scan(a,b) [DVE, full L, no carry], s = h*C_n [DVE/Pool TT],
        # y_psum[db] += I @ s [PE identity matmuls, fp32 accumulation].
        # Evac fuses the silu(z) gate: y = psum * silz [DVE STT].
        NG = 8             # n-group size (psum capacity forces evac-merge)
        NGRP = DS // NG    # 4 groups
        yapool = top.enter_context(tc.tile_pool(name="yapool", bufs=1))
        with ExitStack() as ph:
            repool = ph.enter_context(tc.tile_pool(name="repool", bufs=1))
            dwpool = ph.enter_context(tc.tile_pool(name="dwpool", bufs=2))
            abpool = ph.enter_context(tc.tile_pool(name="abpool", bufs=3))
            hpool = ph.enter_context(tc.tile_pool(name="hpool", bufs=4))
            zpool = ph.enter_context(tc.tile_pool(name="zpool", bufs=2))
            ppsum = ph.enter_context(tc.tile_pool(name="ps6", bufs=2, space="PSUM"))

            y_acc = yapool.tile([P, DB, L], BF16, tag="y_acc")
            for g in range(NGRP):
                ns = slice(g * NG, (g + 1) * NG)
                Bg = repool.tile([P, NG, L], BF16, tag="Bg")
                Cg = repool.tile([P, NG, L], BF16, tag="Cg")
                # g=0: trigger from the idle Pool stream so the loads start
                # as soon as B_d/C_d land, not after the phase-5 Act/SP tails
                eng_ld = nc.gpsimd if g == 0 else (nc.scalar if g % 2 else nc.sync)
                eng_ld.dma_start(
                    Bg[:],
                    B_d[ns, :].rearrange("n t -> (n t)").partition_broadcast(P)
                    .rearrange("p (n t) -> p n t", n=NG))
                (nc.sync if g % 2 else nc.scalar).dma_start(
                    Cg[:],
                    C_d[ns, :].rearrange("n t -> (n t)").partition_broadcast(P)
                    .rearrange("p (n t) -> p n t", n=NG))

                for db in range(DB):
                    dt_q = dwpool.tile([P, L], BF16, tag="dt_q")
                    nc.sync.dma_start(dt_q[:], dt_d[:, db, :])
                    w_q = dwpool.tile([P, L], BF16, tag="w_q")
                    nc.sync.dma_start(w_q[:], w_d[:, db, :])

                    pts = [ppsum.tile([P, TC], F32, name=f"yp{c}_{g}_{db}",
                                      tag=f"yp{c}") for c in range(NTC)]
                    if g == 0:
                        u_q = dwpool.tile([P, L], BF16, tag="u_q")
                        nc.sync.dma_start(u_q[:], u_d[:, db, :])
                        for c in range(NTC):
                            nc.tensor.matmul(pts[c][:], identb[:],
                                             u_q[:, c * TC:(c + 1) * TC],
                                             start=True, stop=False)
                    else:
                        # re-seed psum with the running y_acc partial so the
                        # cross-group merge needs no DVE adds at all
                        for c in range(NTC):
                            nc.tensor.matmul(pts[c][:], identb[:],
                                             y_acc[:, db, c * TC:(c + 1) * TC],
                                             start=True, stop=False)
                    for j in range(NG):
                        n = g * NG + j
                        a_t = abpool.tile([P, L], BF16, tag="a_t")
                        nc.scalar.activation(a_t[:], dt_q[:], AF.Exp,
                                             scale=A_s[:, db, n:n + 1])
                        b_t = abpool.tile([P, L], BF16, tag="b_t")
                        nc.vector.tensor_tensor(b_t[:], Bg[:, j, :], w_q[:], OP.mult)
                        h_t = hpool.tile([P, L], BF16, tag="h_t")
                        nc.vector.tensor_tensor_scan(
                            h_t[:], a_t[:], b_t[:], 0.0, OP.mult, OP.add)
                        s_t = hpool.tile([P, L], BF16, tag="s_t")
                        # s feeds only PE (a latency-tolerant sink): run ~7/8
                        # of them on the otherwise-idle Pool engine.
                        eng_s = nc.vector if (n % 8 == 3) else nc.gpsimd
                        eng_s.tensor_tensor(s_t[:], h_t[:], Cg[:, j, :], OP.mult)
                        for c in range(NTC):
                            nc.tensor.matmul(pts[c][:], identb[:],
                                             s_t[:, c * TC:(c + 1) * TC],
                                             start=False,
                                             stop=(j == NG - 1))
                    # psum already holds the full partial (seeded): evac
                    if g < NGRP - 1:
                        for c in range(NTC):
                            nc.scalar.copy(
                                y_acc[:, db, c * TC:(c + 1) * TC], pts[c][:])
                    else:
                        sz = zpool.tile([P, L], BF16, tag="szg")
                        nc.sync.dma_start(sz[:], silz_d[:, db, :])
                        for c in range(NTC):
                            cs = slice(c * TC, (c + 1) * TC)
                            nc.vector.scalar_tensor_tensor(
                                y_acc[:, db, cs], pts[c][:], 1.0, sz[:, cs],
                                OP.mult, OP.mult)

        # =============== Phase 7: out_proj + stats + LN ===================
        with ExitStack() as ph:
            pool = ph.enter_context(tc.tile_pool(name="p7", bufs=2))
            pool1 = ph.enter_context(tc.tile_pool(name="p7a", bufs=1))
            ph7s = ExitStack()
            ppsum = ph7s.enter_context(tc.tile_pool(name="ps7", bufs=2, space="PSUM"))
            ppsum1 = ph7s.enter_context(tc.tile_pool(name="ps7p", bufs=1, space="PSUM"))

            wout_s = pool1.tile([P, DB, D], BF16, tag="wout_s")
            nc.sync.dma_start(
                wout_s[:], T["w_out_T"][:].rearrange("(db p) o -> p db o", p=P))
            xs_bf = pool1.tile([P, CB, L], BF16, tag="xs_bf")
            for tcn in range(NTC):
                ts_ = slice(tcn * TC, (tcn + 1) * TC)
                for ob in range(CB):
                    pt = ppsum.tile([P, TC], F32, tag="mmo")
                    for db in range(DB):
                        nc.tensor.matmul(pt[:], wout_s[:, db, ob * P:(ob + 1) * P],
                                         y_acc[:, db, ts_],
                                         start=(db == 0), stop=(db == DB - 1))
                    nc.scalar.copy(xs_bf[:, ob, ts_], pt[:])

            st_both_d = dram.tile([1, 2 * L], F32, tag="st_both_d")
            for tcn in range(NTC):
                ts_ = slice(tcn * TC, (tcn + 1) * TC)
                pt = ppsum.tile([1, TC], F32, tag="red2")
                for cb in range(CB):
                    nc.tensor.matmul(pt[:], ones_bf[:], xs_bf[:, cb, ts_],
                                     start=(cb == 0), stop=(cb == CB - 1))
                prow = small.tile([1, TC], F32, tag="prow")
                nc.scalar.copy(prow[:], pt[:])
                nc.sync.dma_start(st_both_d[:, tcn * TC:(tcn + 1) * TC], prow[:])
                pt2 = ppsum.tile([1, TC], F32, tag="red2")
                for cb in range(CB):
                    sqt = pool.tile([P, TC], BF16, tag="sqt2")
                    nc.vector.tensor_tensor(sqt[:], xs_bf[:, cb, ts_],
                                            xs_bf[:, cb, ts_], OP.mult)
                    nc.tensor.matmul(pt2[:], ones_bf[:], sqt[:],
                                     start=(cb == 0), stop=(cb == CB - 1))
                prow2 = small.tile([1, TC], F32, tag="prow")
                nc.scalar.copy(prow2[:], pt2[:])
                nc.sync.dma_start(st_both_d[:, L + tcn * TC:L + (tcn + 1) * TC],
                                  prow2[:])

            LP2 = 2 * LP

            def permute_both(rowd_in_ap, rowd_out_ap, ppsum1):
                # permute BOTH stat halves of a [1, 2L] row in one pass:
                # tiled form [P, (s f)], partition-permute via perm128, then
                # f-permute within each half via blockdiag perm32.
                s_sb = small.tile([P, LP2], F32, tag="perm_in")
                nc.sync.dma_start(
                    s_sb[:, 0:LP],
                    rowd_in_ap[:, 0:L].rearrange("o (p f) -> (o p) f", p=P))
                nc.sync.dma_start(
                    s_sb[:, LP:LP2],
                    rowd_in_ap[:, L:2 * L].rearrange("o (p f) -> (o p) f", p=P))
                pt = ppsum1.tile([P, LP2], F32, tag="permp")
                nc.tensor.matmul(pt[:], perm128_s[:], s_sb[:], start=True, stop=True)
                u_sb = small.tile([P, LP2], F32, tag="perm_u")
                nc.scalar.copy(u_sb[:], pt[:])
                pt2 = ppsum1.tile([LP2, P], F32, tag="permt")
                nc.tensor.transpose(pt2[:], u_sb[:], ident[:])
                ut = small.tile([LP2, P], F32, tag="perm_ut")
                nc.scalar.copy(ut[:], pt2[:])
                pt3 = ppsum1.tile([LP2, P], F32, tag="permt2")
                nc.tensor.matmul(pt3[:], perm32_s[:], ut[:], start=True, stop=True)
                ut2 = small.tile([LP2, P], F32, tag="perm_ut2")
                nc.scalar.copy(ut2[:], pt3[:])
                pt4 = ppsum1.tile([P, LP2], F32, tag="permp2")
                nc.tensor.transpose(pt4[:], ut2[:], ident[0:LP2, 0:LP2])
                s2_sb = small.tile([P, LP2], F32, tag="perm_out")
                nc.scalar.copy(s2_sb[:], pt4[:])
                if rowd_out_ap is None:
                    return s2_sb
                nc.sync.dma_start(
                    rowd_out_ap[:, 0:L].rearrange("o (p f) -> (o p) f", p=P),
                    s2_sb[:, 0:LP])
                nc.sync.dma_start(
                    rowd_out_ap[:, L:2 * L].rearrange("o (p f) -> (o p) f", p=P),
                    s2_sb[:, LP:LP2])

            permute_both(st_both_d, T["cc_in"], ppsum1)
            ph7s.close()  # free stats/permute PSUM banks for the MLP
            if _NO_COLLECTIVE:
                nc.sync.dma_start(T["cc_out"][:], T["cc_in"][:])
            else:
                nc.gpsimd.collective_compute(
                    "AllReduce", OP.add,
                    replica_groups=[[0, 1], [2, 3], [4, 5], [6, 7]],
                    ins=[T["cc_in"][:]], outs=[T["cc_out"][:]],
                )

            # ---- Phase 9 MLP, emitted here so PE/Act work overlaps the ----
            # ---- AllReduce pair-wait (MLP depends only on xn)          ----
            with ExitStack() as ph9:
                pool9 = ph9.enter_context(tc.tile_pool(name="p9", bufs=2))
                pool91 = ph9.enter_context(tc.tile_pool(name="p9a", bufs=1))
                ppsum9 = ph9.enter_context(tc.tile_pool(name="ps9", bufs=2,
                                                        space="PSUM"))
                w1_s = pool91.tile([P, CB, DI], BF16, tag="w1_s")
                nc.sync.dma_start(
                    w1_s[:], T["w1T"][:].rearrange("(cb p) h -> p cb h", p=P))
                w2_s = pool91.tile([P, DB, D], BF16, tag="w2_s")
                nc.sync.dma_start(
                    w2_s[:], T["w2T"][:].rearrange("(db p) o -> p db o", p=P))
                xn_bf = pool91.tile([P, CB, L], BF16, tag="xn_bf")
                nc.sync.dma_start(xn_bf[:], xn_bf_d[:])
                g_bf = pool91.tile([P, DB, L], BF16, tag="g_bf")
                for tcn in range(NTC):
                    ts_ = slice(tcn * TC, (tcn + 1) * TC)
                    for hb in range(DB):
                        pt = ppsum9.tile([P, TC], F32, tag="mm1")
                        for cb in range(CB):
                            nc.tensor.matmul(
                                pt[:], w1_s[:, cb, hb * P:(hb + 1) * P],
                                xn_bf[:, cb, ts_],
                                start=(cb == 0), stop=(cb == CB - 1))
                        nc.scalar.activation(g_bf[:, hb, ts_], pt[:], AF.Gelu,
                                             bias=b1_s[:, hb:hb + 1])
                    for ob in range(CB):
                        pt = ppsum9.tile([P, TC], F32, tag="mm2")
                        for hb in range(DB):
                            nc.tensor.matmul(
                                pt[:], w2_s[:, hb, ob * P:(ob + 1) * P],
                                g_bf[:, hb, ts_],
                                start=(hb == 0), stop=(hb == DB - 1))
                        ot = pool9.tile([P, TC], F32, tag="oml")
                        nc.scalar.copy(ot[:], pt[:])
                        nc.sync.dma_start(T["out_mlp"][ob * P:(ob + 1) * P, ts_],
                                          ot[:])

            # ---- post-collective LN over the concatenated dirs ----
            ph7t = ExitStack()
            ppsum2 = ph7t.enter_context(tc.tile_pool(name="ps7q", bufs=1,
                                                     space="PSUM"))
            both3 = permute_both(T["cc_out"], None, ppsum2)

            mu3 = both3[:, 0:LP]
            v3 = both3[:, LP:LP2]
            nc.vector.tensor_scalar_mul(mu3, mu3, 1.0 / (2 * D))
            nc.vector.tensor_scalar_mul(v3, v3, 1.0 / (2 * D))
            mu32 = small.tile([P, LP], F32, tag="mu32")
            nc.vector.tensor_tensor(mu32[:], mu3, mu3, OP.mult)
            nc.vector.tensor_sub(v3, v3, mu32[:])
            nc.vector.tensor_scalar_add(v3, v3, 1e-5)
            nc.scalar.sqrt(v3, v3)
            nc.vector.reciprocal(v3, v3)
            nc.vector.tensor_tensor(mu32[:], mu3, v3, OP.mult)
            mr2_rep = pool1.tile([P, L], BF16, tag="mr2_rep")
            rstd2_rep = pool1.tile([P, L], BF16, tag="rstd2_rep")
            replicate_tiled_bf(mu32, mr2_rep, "ln2m")
            replicate_tiled_bf_ap(v3, rstd2_rep, "ln2v")

            xs_ln = pool1.tile([P, CB, L], BF16, tag="xs_ln")
            for cb in range(CB):
                u = pool.tile([P, L], BF16, tag="u_ln2")
                nc.vector.tensor_tensor(u[:], xs_bf[:, cb, :], rstd2_rep[:], OP.mult)
                nc.vector.tensor_sub(u[:], u[:], mr2_rep[:])
                nc.vector.tensor_scalar(xs_ln[:, cb, :], u[:],
                                        lnp_w_s[:, cb:cb + 1],
                                        lnp_b_s[:, cb:cb + 1], OP.mult, OP.add)

            ph7t.close()
            # ---- Phase 8 mixer conv half (PE diag matmuls) ----
            with ExitStack() as ph8:
                pool8 = ph8.enter_context(tc.tile_pool(name="p8", bufs=2))
                pool81 = ph8.enter_context(tc.tile_pool(name="p8a", bufs=1))
                ppsum8 = ph8.enter_context(tc.tile_pool(name="ps8", bufs=4,
                                                        space="PSUM"))
                # padded E/O slab [p, eo, gh, t]; SBUF->SBUF interleave
                # gather: EO[p',eo,gh] = xs_ln channel gh*256 + 2p' + eo
                EO = pool81.tile([P, 2, 2, 2 + L], BF16, tag="EO")
                nc.vector.memset(EO[:, :, :, 0:1], 0.0)
                nc.vector.memset(EO[:, :, :, 1 + L:2 + L], 0.0)
                slab_r = xs_ln[:].rearrange("(a two) cb t -> a two cb t", two=2)
                for eo in range(2):
                    for gh in range(2):
                        for half in range(2):
                            nc.sync.dma_start(
                                EO[half * 64:(half + 1) * 64, eo, gh, 1:1 + L],
                                slab_r[:, eo, gh * 2 + half, :])
                diag_pc = pool81.tile([P, 2, 6, P], BF16, tag="diag_pc")
                for gh in range(2):
                    for i in range(6):
                        nc.vector.tensor_scalar(
                            diag_pc[:, gh, i, :], identb[:], pc_w_s[:, gh, i:i + 1],
                            None, OP.mult)
                for gh in range(2):
                    for tcn in range(NTC):
                        pt = ppsum8.tile([P, TC], F32, tag="mxpsum")
                        for eo in range(2):
                            for k in range(3):
                                nc.tensor.matmul(
                                    pt[:], diag_pc[:, gh, eo * 3 + k, :],
                                    EO[:, eo, gh, k + tcn * TC:k + (tcn + 1) * TC],
                                    start=(eo == 0 and k == 0),
                                    stop=(eo == 1 and k == 2))
                        mout = pool8.tile([P, TC], F32, tag="mout")
                        nc.scalar.activation(mout[:], pt[:], AF.Silu,
                                             bias=pc_b_s[:, gh:gh + 1])
                        nc.sync.dma_start(
                            T["out_mix"][gh * P:(gh + 1) * P,
                                         tcn * TC:(tcn + 1) * TC],
                            mout[:])


_NC_CACHE = None


def _get_nc():
    global _NC_CACHE
    if _NC_CACHE is None:
        _NC_CACHE = _build()
    return _NC_CACHE


def _prep_core_inputs(inputs, b, rev):
    import ml_dtypes
    f32 = np.float32
    bf16 = ml_dtypes.bfloat16

    def dpart(v, nb):  # [nb*128, ...] -> [128, nb, ...]
        v = np.asarray(v, dtype=f32)
        return np.ascontiguousarray(
            v.reshape(nb, P, *v.shape[1:]).transpose(1, 0, *range(2, v.ndim + 1)))

    x = inputs["x"][b]
    if rev:
        x = x[::-1]
    lc_w = inputs["lc_w"][:, 0, :]
    if rev:
        lc_w = lc_w[:, ::-1]
    lnp_w = inputs["lnp_w"][rev * D:(rev + 1) * D]
    lnp_b = inputs["lnp_b"][rev * D:(rev + 1) * D]
    pc_w = inputs["pc_w"][rev * (D // 2):(rev + 1) * (D // 2)]
    if rev:
        pc_w = pc_w[:, :, ::-1]
    pc_b = inputs["pc_b"][rev * (D // 2):(rev + 1) * (D // 2)]
    hsl = slice(rev * DI, (rev + 1) * DI)
    w1 = inputs["w1"][hsl]
    b1v = inputs["b1"][hsl]
    w2 = inputs["w2"][:, hsl]
    A = -np.exp(inputs["A_log"].astype(np.float64)).astype(f32)
    eye = np.eye(P, dtype=f32)
    rv = np.ascontiguousarray(np.eye(P, dtype=f32)[::-1])
    e16 = np.eye(LP, dtype=f32)
    r16 = np.ascontiguousarray(e16[::-1])
    # blockdiag: f-reversal applied independently to the two stat halves
    # of the [1, 2L] row (tiled as [P, (s f)] -> transposed [(s f), P])
    e32 = np.eye(2 * LP, dtype=f32)
    r32 = np.zeros((2 * LP, 2 * LP), f32)
    r32[0:LP, 0:LP] = r16
    r32[LP:, LP:] = r16

    return {
        "x_seq": np.ascontiguousarray(x, dtype=f32),
        "w_in_T": np.ascontiguousarray(inputs["in_w"].astype(f32).T.astype(bf16)),
        "lc_w": dpart(lc_w, CB),
        "lc_b": dpart(inputs["lc_b"], CB),
        "norm_w": dpart(inputs["norm_w"], CB),
        "lnc_w": dpart(inputs["lnc_w"], CB),
        "lnc_b": dpart(inputs["lnc_b"], CB),
        "cv_w": dpart(inputs["cv_w"][:, 0, :], DB),
        "cv_b": dpart(inputs["cv_b"], DB),
        "xp_wT": np.ascontiguousarray(inputs["xp_w"].astype(f32).T.astype(bf16)),
        "dtp_wT": np.ascontiguousarray(inputs["dtp_w"].astype(f32).T.astype(bf16)),
        "dtp_b": dpart(inputs["dtp_b"], DB),
        "A_dn": dpart(A, DB),
        "Dp_dn": dpart(inputs["Dp"], DB),
        "w_out_T": np.ascontiguousarray(inputs["out_w"].astype(f32).T.astype(bf16)),
        "lnp_w": dpart(lnp_w, CB),
        "lnp_b": dpart(lnp_b, CB),
        "pc_w": dpart(np.ascontiguousarray(pc_w).reshape(D // 2, 6), 2),
        "pc_b": dpart(pc_b, 2),
        "w1T": np.ascontiguousarray(np.asarray(w1, dtype=f32).T.astype(bf16)),
        "b1": dpart(b1v, DB),
        "w2T": np.ascontiguousarray(np.asarray(w2, dtype=f32).T.astype(bf16)),
        "perm128": rv if rev else eye,
        "perm32": r32 if rev else e32,
    }


LAST_RESULTS = None


def kernel(**inputs):
    global LAST_RESULTS
    inputs = {k: np.asarray(v) for k, v in inputs.items()}
    nc = _get_nc()
    in_maps = [_prep_core_inputs(inputs, c // 2, c % 2) for c in range(8)]
    res = run_bass_kernel_spmd(nc, in_maps, core_ids=list(range(8)))
    LAST_RESULTS = res
    out = np.empty((B, L, D), np.float32)
    b2 = inputs["b2"].astype(np.float32)
    for b in range(B):
        mf = res.results[2 * b]
        mb = res.results[2 * b + 1]
        acc = inputs["x"][b].astype(np.float32) + b2[None, :]
        acc += mf["out_mlp"].T
        acc += mb["out_mlp"][:, ::-1].T
        acc[:, 0:D // 2] += mf["out_mix"].T
        acc[:, D // 2:] += mb["out_mix"][:, ::-1].T
        out[b] = acc
    return out



# revision 63
# speedup vs baseline: 1.1668x; 1.1590x over previous
"""Bidirectional Conv-Mamba block on 8 Trainium2 NeuronCores.

Sharding: core c = (b = c//2, dir = c%2). Each core runs the full mamba for
its (sample, direction) on a direction-local (possibly reversed) sequence,
plus the direction's half of the tail (mixer conv channel-half + MLP
ffn-half; the pc-conv groups do not mix directions). The only cross-core
exchange is the post-concat LayerNorm sum/sumsq stats: a [2*L] f32
AllReduce between pair cores, with time alignment handled by per-core
input permutation matrices. Host sums the partial outputs during unshard.

Engine placement (per core), tuned against the TRN2 cost model:
- Selective scan: full-length (L=2048, carry-free) tensor_tensor_scan per
  (d-block, state) on DVE; b = w*B_n on DVE (bf16 2x mode); s = h*C_n
  mostly on the Pool engine (7/8), which walrus allows for TensorTensor.
- Readout sum over states: PE identity-matmul accumulation in fp32 PSUM,
  seeded per n-group with I@y_acc (and I@u at g=0) so cross-group merges
  are plain Act copies; the final evac fuses the silu(z) gate (DVE STT).
- All depthwise convs (lc 3-tap, cv 4-tap causal, pc 2x3-tap) run as
  PE diagonal-stationary matmuls accumulating shifted slices in PSUM.
- rmsnorm stats via bn_stats on the [t-rows, d] layout; LN applies in
  bf16 (2x/4x DVE modes); x transposed via XBAR dma_start_transpose.
- The MLP is emitted between the stats AllReduce and the post-permute so
  its PE/Act work hides the collective pair-wait.
"""

import numpy as np

import concourse.bass as bass
import concourse.mybir as mybir
import concourse.tile as tile
from concourse.bass_utils import run_bass_kernel_spmd

F32 = mybir.dt.float32
BF16 = mybir.dt.bfloat16
AF = mybir.ActivationFunctionType
OP = mybir.AluOpType

B, L, D = 4, 2048, 512
DI, DS, DTR, K4 = 1024, 32, 32, 4
_NO_COLLECTIVE = False
P = 128
CB = D // P          # 4 col-blocks of D
DB = DI // P         # 8 d-blocks of DI
TC = 512             # matmul t-chunk
NTC = L // TC
LP = L // P          # 16


def _split_excess_waits(nc):
    """This toolchain's walrus accepts at most one semaphore wait per
    instruction; hoist extra waits onto NoOp carriers placed just before."""
    for f in nc.m.functions:
        for blk in f.blocks:
            insts = blk.instructions  # live list
            i = 0
            k = 0
            while i < len(insts):
                inst = insts[i]
                si = getattr(inst, "sync_info", None)
                if si is not None and si.on_wait and len(si.on_wait) > 1:
                    waits = list(si.on_wait)
                    for w in waits[:-1]:
                        nop = mybir.InstNoOp(name=f"wc{k}_{inst.name}", ins=[], outs=[])
                        nop.engine = inst.engine
                        nop.sync_info = mybir.SyncInfo(on_wait=[w], on_update=[])
                        insts.insert(i, nop)
                        i += 1
                        k += 1
                    inst.sync_info = mybir.SyncInfo(
                        on_wait=[waits[-1]], on_update=list(si.on_update)
                    )
                i += 1


def _build():
    nc = bass.Bass("TRN2", num_devices=8)

    di = lambda n, s: nc.dram_tensor(n, s, F32, kind="ExternalInput")
    dib = lambda n, s: nc.dram_tensor(n, s, BF16, kind="ExternalInput")

    T = {}
    T["x_seq"] = di("x_seq", [L, D])
    T["w_in_T"] = dib("w_in_T", [D, 2 * DI])
    T["lc_w"] = di("lc_w", [P, CB, 3])
    T["lc_b"] = di("lc_b", [P, CB])
    T["norm_w"] = di("norm_w", [P, CB])
    T["lnc_w"] = di("lnc_w", [P, CB])
    T["lnc_b"] = di("lnc_b", [P, CB])
    T["cv_w"] = di("cv_w", [P, DB, K4])
    T["cv_b"] = di("cv_b", [P, DB])
    T["xp_wT"] = dib("xp_wT", [DI, DTR + 2 * DS])
    T["dtp_wT"] = dib("dtp_wT", [DTR, DI])
    T["dtp_b"] = di("dtp_b", [P, DB])
    T["A_dn"] = di("A_dn", [P, DB, DS])
    T["Dp_dn"] = di("Dp_dn", [P, DB])
    T["w_out_T"] = dib("w_out_T", [DI, D])
    T["lnp_w"] = di("lnp_w", [P, CB])
    T["lnp_b"] = di("lnp_b", [P, CB])
    T["pc_w"] = di("pc_w", [P, 2, 6])   # [g, gh, i*3+k]
    T["pc_b"] = di("pc_b", [P, 2])
    T["w1T"] = dib("w1T", [D, DI])      # ffn half
    T["b1"] = di("b1", [P, DB])
    T["w2T"] = dib("w2T", [DI, D])
    T["perm128"] = di("perm128", [P, P])
    T["perm32"] = di("perm32", [2 * LP, 2 * LP])

    T["out_mlp"] = nc.dram_tensor("out_mlp", [D, L], F32, kind="ExternalOutput")
    T["out_mix"] = nc.dram_tensor("out_mix", [D // 2, L], F32, kind="ExternalOutput")

    T["cc_in"] = nc.dram_tensor("cc_in", [1, 2 * L], F32)
    T["cc_out"] = nc.dram_tensor("cc_out", [1, 2 * L], F32)

    with tile.TileContext(nc) as tc:
        _emit(nc, tc, T)

    _split_excess_waits(nc)
    return nc


def _emit(nc, tc, T):
    from contextlib import ExitStack
    from concourse.masks import make_identity

    TS = 512           # scan time-chunk
    NQ = L // TS       # 4

    with ExitStack() as top:
        consts = top.enter_context(tc.tile_pool(name="consts", bufs=1))
        small = top.enter_context(tc.tile_pool(name="small", bufs=2))
        dram = top.enter_context(tc.tile_pool(name="dram", bufs=2, space="PSUM" if False else "DRAM"))

        def cload(name):
            src = T[name][:]
            t = consts.tile(list(src.shape), src.dtype, tag=f"c_{name}")
            nc.scalar.dma_start(t[:], src)
            return t

        lc_w_s = cload("lc_w"); lc_b_s = cload("lc_b")
        norm_w_s = cload("norm_w")
        lnc_w_s = cload("lnc_w"); lnc_b_s = cload("lnc_b")
        cv_w_s = cload("cv_w"); cv_b_s = cload("cv_b")
        dtp_b_s = cload("dtp_b"); A_s = cload("A_dn"); Dp_s = cload("Dp_dn")
        lnp_w_s = cload("lnp_w"); lnp_b_s = cload("lnp_b")
        pc_w_s = cload("pc_w"); pc_b_s = cload("pc_b")
        b1_s = cload("b1")
        perm128_s = cload("perm128"); perm32_s = cload("perm32")

        ident = consts.tile([P, P], F32, tag="ident")
        make_identity(nc, ident[:])
        identb = consts.tile([P, P], BF16, tag="identb")
        nc.vector.tensor_copy(identb[:], ident[:])
        ones_bf = consts.tile([P, 1], BF16, tag="ones_bf")
        nc.gpsimd.memset(ones_bf[:], 1.0)

        def replicate_rowd(rowd, dst_PL):
            nc.sync.dma_start(
                dst_PL[:], rowd[:].rearrange("o t -> (o t)").partition_broadcast(P))

        def rowd_to_tiled(rowd_ap, dst_sb):
            nc.sync.dma_start(dst_sb[:], rowd_ap.rearrange("o (p f) -> (o p) f", p=P))

        def rsqrt_tile(v):
            nc.scalar.sqrt(v[:], v[:])
            nc.vector.reciprocal(v[:], v[:])

        def replicate_tiled_bf_ap(src_f32_ap, dst_PL_bf, tagp):
            # f32 [P,LP] tiled stat -> bf16 row -> partition-broadcast [P,L]
            b = small.tile([P, LP], BF16, name=f"{tagp}_b", tag=f"{tagp}_b")
            nc.vector.tensor_copy(b[:], src_f32_ap)
            rowd = dram.tile([1, L], BF16, name=f"{tagp}_rd", tag=f"{tagp}_rd")
            nc.sync.dma_start(rowd[:].rearrange("o (p f) -> (o p) f", p=P), b[:])
            nc.sync.dma_start(
                dst_PL_bf[:],
                rowd[:].rearrange("o t -> (o t)").partition_broadcast(P))

        def replicate_tiled_bf(src_f32_sb, dst_PL_bf, tagp):
            replicate_tiled_bf_ap(src_f32_sb[:], dst_PL_bf, tagp)

        # =============== Phase 0-2: xn, xc, ssm_in ========================
        sA = ExitStack()  # ssm_bf: lives to end of in_proj
        ssm_pool = sA.enter_context(tc.tile_pool(name="ssm_pool", bufs=1))
        ssm_bf = ssm_pool.tile([P, CB, L], BF16, tag="ssm_bf")
        xn_bf_d = dram.tile([P, CB, L], BF16, tag="xn_spill")
        with ExitStack() as ph:
            pool = ph.enter_context(tc.tile_pool(name="p02", bufs=2))
            pool1 = ph.enter_context(tc.tile_pool(name="p02a", bufs=1))
            ppsum = ph.enter_context(tc.tile_pool(name="ps02", bufs=2, space="PSUM"))

            # padded bf16 xn slab: [:, cb, 1:1+L] is xn; edges zero for conv3
            xn_bp = pool1.tile([P, CB, 2 + L], BF16, tag="xn_bp")
            nc.vector.memset(xn_bp[:, :, 0:1], 0.0)
            nc.vector.memset(xn_bp[:, :, 1 + L:2 + L], 0.0)
            with ExitStack() as ph2:
                pool2 = ph2.enter_context(tc.tile_pool(name="p02b", bufs=1))
                xrows = pool2.tile([P, LP, D], F32, tag="xrows")
                H = LP // 2
                for hh in range(2):
                    nc.sync.dma_start(
                        xrows[:, hh * H:(hh + 1) * H, :],
                        T["x_seq"][hh * H * P:(hh + 1) * H * P, :]
                        .rearrange("(tt p) d -> p tt d", p=P))
                # bf16 copy (Act, overlaps DVE bn_stats), then XBAR DMA
                # transposes replace the PE transpose + evac pipeline
                xrows_bf = pool2.tile([P, LP, D], BF16, tag="xrows_bf")
                for hh in range(2):
                    nc.scalar.copy(xrows_bf[:, hh * H:(hh + 1) * H, :],
                                   xrows[:, hh * H:(hh + 1) * H, :])
                x_d = pool2.tile([P, CB, L], BF16, tag="x_d")
                for tt in range(LP):
                    nc.sync.dma_start_transpose(
                        x_d[:, :, tt * P:(tt + 1) * P], xrows_bf[:, tt, :])
                # rmsnorm: mean(x^2) over d via bn_stats on the row layout;
                # result lands f-major tiled: msq[p, tt] = stat[t=tt*128+p]
                stat6 = pool.tile([P, LP, 6], F32, tag="stat6")
                mv = pool.tile([P, LP, 2], F32, tag="mv")
                for tt in range(LP):
                    nc.vector.bn_stats(stat6[:, tt, :], xrows[:, tt, :])
                    nc.vector.bn_aggr(mv[:, tt, :], stat6[:, tt, :])
                msq = small.tile([P, LP], F32, tag="msq")
                nc.vector.tensor_tensor(msq[:], mv[:, :, 0], mv[:, :, 0], OP.mult)
                nc.vector.tensor_add(msq[:], msq[:], mv[:, :, 1])
                nc.vector.tensor_scalar_add(msq[:], msq[:], 1.1920929e-07)
                rsqrt_tile(msq)
                ms_row_d = dram.tile([1, L], F32, tag="ms_row_d")
                nc.sync.dma_start(
                    ms_row_d[:].rearrange("o (f p) -> (o p) f", p=P), msq[:])
                rs_rep = pool.tile([P, L], F32, tag="rs_rep")
                replicate_rowd(ms_row_d, rs_rep)
                for cb in range(CB):
                    nc.vector.scalar_tensor_tensor(
                        xn_bp[:, cb, 1:1 + L], x_d[:, cb, :], norm_w_s[:, cb:cb + 1],
                        rs_rep[:], OP.mult, OP.mult)

            # conv3 via PE diag matmuls; xc = conv result + bias (bf16)
            diag_lc = pool1.tile([P, CB, 3, P], BF16, tag="diag_lc")
            for cb in range(CB):
                for k in range(3):
                    nc.vector.tensor_scalar(
                        diag_lc[:, cb, k, :], identb[:], lc_w_s[:, cb, k:k + 1],
                        None, OP.mult)
            xc = pool1.tile([P, CB, L], BF16, tag="xc")
            for cb in range(CB):
                for tcn in range(NTC):
                    pt = ppsum.tile([P, TC], F32, tag="c3psum")
                    for k in range(3):
                        nc.tensor.matmul(
                            pt[:], diag_lc[:, cb, k, :],
                            xn_bp[:, cb, k + tcn * TC:k + (tcn + 1) * TC],
                            start=(k == 0), stop=(k == 2))
                    nc.scalar.activation(xc[:, cb, tcn * TC:(tcn + 1) * TC],
                                         pt[:], AF.Identity,
                                         bias=lc_b_s[:, cb:cb + 1])
            # LN over D
            mu_row_d = dram.tile([1, L], F32, tag="mu_row_d")
            ms2_row_d = dram.tile([1, L], F32, tag="ms2_row_d")
            for tcn in range(NTC):
                ts_ = slice(tcn * TC, (tcn + 1) * TC)
                pt = ppsum.tile([1, TC], F32, tag="red")
                for cb in range(CB):
                    nc.tensor.matmul(pt[:], ones_bf[:], xc[:, cb, ts_],
                                     start=(cb == 0), stop=(cb == CB - 1))
                prow = small.tile([1, TC], F32, tag="prow")
                nc.scalar.copy(prow[:], pt[:])
                nc.sync.dma_start(mu_row_d[:, ts_], prow[:])
                pt2 = ppsum.tile([1, TC], F32, tag="red")
                for cb in range(CB):
                    sqt = pool.tile([P, TC], BF16, tag="sqt")
                    nc.vector.tensor_tensor(sqt[:], xc[:, cb, ts_], xc[:, cb, ts_],
                                            OP.mult)
                    nc.tensor.matmul(pt2[:], ones_bf[:], sqt[:],
                                     start=(cb == 0), stop=(cb == CB - 1))
                prow2 = small.tile([1, TC], F32, tag="prow")
                nc.vector.tensor_copy(prow2[:], pt2[:])
                nc.sync.dma_start(ms2_row_d[:, ts_], prow2[:])
            mu_sb = small.tile([P, LP], F32, tag="mu_sb")
            v_sb = small.tile([P, LP], F32, tag="v_sb")
            rowd_to_tiled(mu_row_d[:], mu_sb)
            rowd_to_tiled(ms2_row_d[:], v_sb)
            nc.vector.tensor_scalar_mul(mu_sb[:], mu_sb[:], 1.0 / D)
            nc.vector.tensor_scalar_mul(v_sb[:], v_sb[:], 1.0 / D)
            mu2 = small.tile([P, LP], F32, tag="mu2")
            nc.vector.tensor_tensor(mu2[:], mu_sb[:], mu_sb[:], OP.mult)
            nc.vector.tensor_sub(v_sb[:], v_sb[:], mu2[:])
            nc.vector.tensor_scalar_add(v_sb[:], v_sb[:], 1e-5)
            rsqrt_tile(v_sb)
            nc.vector.tensor_tensor(mu2[:], mu_sb[:], v_sb[:], OP.mult)
            mr_rep = pool1.tile([P, L], BF16, tag="mr_rep")
            rstd_rep = pool1.tile([P, L], BF16, tag="rstd_rep")
            replicate_tiled_bf(mu2, mr_rep, "ln1m")
            replicate_tiled_bf(v_sb, rstd_rep, "ln1v")
            for cb in range(CB):
                u = pool.tile([P, L], BF16, tag="u_ln")
                nc.vector.tensor_tensor(u[:], xc[:, cb, :], rstd_rep[:], OP.mult)
                nc.vector.tensor_sub(u[:], u[:], mr_rep[:])
                nc.vector.tensor_scalar(u[:], u[:], lnc_w_s[:, cb:cb + 1],
                                        lnc_b_s[:, cb:cb + 1], OP.mult, OP.add)
                nc.scalar.activation(u[:], u[:], AF.Silu)
                nc.vector.tensor_tensor(ssm_bf[:, cb, :], u[:],
                                        xn_bp[:, cb, 1:1 + L], OP.add)
                nc.sync.dma_start(xn_bf_d[:, cb, :], xn_bp[:, cb, 1:1 + L])

        # =============== Phase 3: in_proj =================================
        silz_d = dram.tile([P, DB, L], BF16, tag="silz_spill")
        sB = ExitStack()  # xmpre: lives to end of conv4
        xmp_pool = sB.enter_context(tc.tile_pool(name="xmp_pool", bufs=1, side="right"))
        xmpre = xmp_pool.tile([P, DB, 3 + L], BF16, tag="xmpre")
        with ExitStack() as ph:
            pool = ph.enter_context(tc.tile_pool(name="p3", bufs=2))
            pool1 = ph.enter_context(tc.tile_pool(name="p3a", bufs=1))
            ppsum = ph.enter_context(tc.tile_pool(name="ps3", bufs=2, space="PSUM"))
            w_in_s = pool1.tile([P, CB, 2 * DI], BF16, tag="w_in_s")
            nc.sync.dma_start(
                w_in_s[:], T["w_in_T"][:].rearrange("(cb p) j -> p cb j", p=P))
            nc.vector.memset(xmpre[:, :, 0:3], 0.0)
            # jb-outer with the xm blocks first: each xmpre[db] completes
            # early so conv4 pipelines with the rest of in_proj; silz last
            for jb in range(2 * DB):
                for tcn in range(NTC):
                    ts_ = slice(tcn * TC, (tcn + 1) * TC)
                    pt = ppsum.tile([P, TC], F32, tag="mmj")
                    for cb in range(CB):
                        nc.tensor.matmul(pt[:], w_in_s[:, cb, jb * P:(jb + 1) * P],
                                         ssm_bf[:, cb, ts_],
                                         start=(cb == 0), stop=(cb == CB - 1))
                    if jb < DB:
                        nc.vector.tensor_copy(
                            xmpre[:, jb, 3 + tcn * TC:3 + (tcn + 1) * TC], pt[:])
                    else:
                        sz = pool.tile([P, TC], BF16, tag="sz")
                        nc.scalar.activation(sz[:], pt[:], AF.Silu)
                        nc.sync.dma_start(silz_d[:, jb - DB, ts_], sz[:])
        sA.close()  # free ssm_bf

        # =============== Phase 4: conv4 via PE diag matmuls ===============
        sX = ExitStack()  # xm_bf: lives to end of phase 5
        xm_pool = sX.enter_context(tc.tile_pool(name="xm_pool", bufs=1))
        xm_bf = xm_pool.tile([P, DB, L], BF16, tag="xm_bf")
        with ExitStack() as ph:
            pool1 = ph.enter_context(tc.tile_pool(name="p4a", bufs=1))
            ppsum = ph.enter_context(tc.tile_pool(name="ps4", bufs=4, space="PSUM"))
            # diag stationaries: diag(cv_w[:, db, k]) = identity * w (per-row)
            diag_cv = pool1.tile([P, DB, K4, P], BF16, tag="diag_cv")
            for db in range(DB):
                for k in range(K4):
                    nc.vector.tensor_scalar(
                        diag_cv[:, db, k, :], identb[:], cv_w_s[:, db, k:k + 1],
                        None, OP.mult)
            for db in range(DB):
                for tcn in range(NTC):
                    pt = ppsum.tile([P, TC], F32, tag="cpsum")
                    for k in range(K4):
                        nc.tensor.matmul(
                            pt[:], diag_cv[:, db, k, :],
                            xmpre[:, db, k + tcn * TC:k + (tcn + 1) * TC],
                            start=(k == 0), stop=(k == K4 - 1))
                    nc.scalar.activation(xm_bf[:, db, tcn * TC:(tcn + 1) * TC],
                                         pt[:], AF.Silu, bias=cv_b_s[:, db:db + 1])
        sB.close()  # free xmpre

        # =============== Phase 5: projections =============================
        dt_d = dram.tile([P, DB, L], BF16, tag="dt_spill")
        w_d = dram.tile([P, DB, L], BF16, tag="w_spill")
        u_d = dram.tile([P, DB, L], BF16, tag="u_spill")      # Dp*xm
        B_d = dram.tile([DS, L], BF16, tag="B_d")
        C_d = dram.tile([DS, L], BF16, tag="C_d")
        with ExitStack() as ph:
            pool = ph.enter_context(tc.tile_pool(name="p45", bufs=2))
            pool1 = ph.enter_context(tc.tile_pool(name="p45a", bufs=1))
            ppsum = ph.enter_context(tc.tile_pool(name="ps45", bufs=2, space="PSUM"))

            xp_s = pool1.tile([P, DB, DTR + 2 * DS], BF16, tag="xp_s")
            nc.sync.dma_start(
                xp_s[:], T["xp_wT"][:].rearrange("(db p) j -> p db j", p=P))
            dtp_s = pool1.tile([DTR, DI], BF16, tag="dtp_s")
            nc.sync.dma_start(dtp_s[:], T["dtp_wT"][:])
            dtpre = pool1.tile([DTR, L], BF16, tag="dtpre")
            B_bf = pool1.tile([DS, L], BF16, tag="B_bf")
            C_bf = pool1.tile([DS, L], BF16, tag="C_bf")
            for tcn in range(NTC):
                ts_ = slice(tcn * TC, (tcn + 1) * TC)
                pt = ppsum.tile([DTR + 2 * DS, TC], F32, tag="mmxp")
                for db in range(DB):
                    nc.tensor.matmul(pt[:], xp_s[:, db, :], xm_bf[:, db, ts_],
                                     start=(db == 0), stop=(db == DB - 1))
                nc.scalar.copy(dtpre[:, ts_], pt[0:DTR, :])
                nc.scalar.copy(B_bf[:, ts_], pt[DTR:DTR + DS, :])
                nc.scalar.copy(C_bf[:, ts_], pt[DTR + DS:, :])
            nc.sync.dma_start(B_d[:], B_bf[:])
            nc.sync.dma_start(C_d[:], C_bf[:])
            # db-outer so phase 6 (which consumes per-db spills) can start on
            # db=0 while later dbs are still being produced
            for db in range(DB):
                for tcn in range(NTC):
                    ts_ = slice(tcn * TC, (tcn + 1) * TC)
                    pt = ppsum.tile([P, TC], F32, tag="mmdt")
                    nc.tensor.matmul(pt[:], dtp_s[:, db * P:(db + 1) * P],
                                     dtpre[:, ts_], start=True, stop=True)
                    ett = pool.tile([P, TC], F32, tag="ett")
                    nc.scalar.activation(ett[:], pt[:], AF.Exp,
                                         bias=dtp_b_s[:, db:db + 1])
                    dtt = pool.tile([P, TC], BF16, tag="dtt")
                    nc.scalar.activation(dtt[:], ett[:], AF.Ln, bias=1.0)
                    nc.sync.dma_start(dt_d[:, db, ts_], dtt[:])
                    wt = pool.tile([P, TC], BF16, tag="wt")
                    nc.vector.tensor_tensor(wt[:], dtt[:], xm_bf[:, db, ts_], OP.mult)
                    nc.sync.dma_start(w_d[:, db, ts_], wt[:])
                ut = pool.tile([P, L], BF16, tag="ut")
                nc.vector.tensor_scalar(ut[:], xm_bf[:, db, :],
                                        Dp_s[:, db:db + 1], None, OP.mult)
                nc.sync.dma_start(u_d[:, db, :], ut[:])
        sX.close()  # free xm_bf

        # =============== Phase 6: full-length selective scan ==============
        # Per (db, n): a = exp(A*dt) [Act], b = w*B_n [DVE/Pool TT],
        # h = scan(a,b) [DVE, full L, no carry], s = h*C_n [DVE/Pool TT],
        # y_psum[db] += I @ s [PE identity matmuls, fp32 accumulation].
        # Evac fuses the silu(z) gate: y = psum * silz [DVE STT].
        NG = 8             # n-group size (psum capacity forces evac-merge)
        NGRP = DS // NG    # 4 groups
        yapool = top.enter_context(tc.tile_pool(name="yapool", bufs=1))
        with ExitStack() as ph:
            repool = ph.enter_context(tc.tile_pool(name="repool", bufs=1))
            dwpool = ph.enter_context(tc.tile_pool(name="dwpool", bufs=2))
            abpool = ph.enter_context(tc.tile_pool(name="abpool", bufs=3))
            hpool = ph.enter_context(tc.tile_pool(name="hpool", bufs=4))
            zpool = ph.enter_context(tc.tile_pool(name="zpool", bufs=2))
            ppsum = ph.enter_context(tc.tile_pool(name="ps6", bufs=2, space="PSUM"))

            y_acc = yapool.tile([P, DB, L], BF16, tag="y_acc")
            for g in range(NGRP):
                ns = slice(g * NG, (g + 1) * NG)
                Bg = repool.tile([P, NG, L], BF16, tag="Bg")
                Cg = repool.tile([P, NG, L], BF16, tag="Cg")
                # g=0: trigger from the idle Pool stream so the loads start
                # as soon as B_d/C_d land, not after the phase-5 Act/SP tails
                eng_ld = nc.gpsimd if g == 0 else (nc.scalar if g % 2 else nc.sync)
                eng_ld.dma_start(
                    Bg[:],
                    B_d[ns, :].rearrange("n t -> (n t)").partition_broadcast(P)
                    .rearrange("p (n t) -> p n t", n=NG))
                (nc.sync if g % 2 else nc.scalar).dma_start(
                    Cg[:],
                    C_d[ns, :].rearrange("n t -> (n t)").partition_broadcast(P)
                    .rearrange("p (n t) -> p n t", n=NG))

                for db in range(DB):
                    dt_q = dwpool.tile([P, L], BF16, tag="dt_q")
                    nc.sync.dma_start(dt_q[:], dt_d[:, db, :])
                    w_q = dwpool.tile([P, L], BF16, tag="w_q")
                    nc.sync.dma_start(w_q[:], w_d[:, db, :])

                    pts = [ppsum.tile([P, TC], F32, name=f"yp{c}_{g}_{db}",
                                      tag=f"yp{c}") for c in range(NTC)]
                    if g == 0:
                        u_q = dwpool.tile([P, L], BF16, tag="u_q")
                        nc.sync.dma_start(u_q[:], u_d[:, db, :])
                        for c in range(NTC):
                            nc.tensor.matmul(pts[c][:], identb[:],
                                             u_q[:, c * TC:(c + 1) * TC],
                                             start=True, stop=False)
                    else:
                        # re-seed psum with the running y_acc partial so the
                        # cross-group merge needs no DVE adds at all
                        for c in range(NTC):
                            nc.tensor.matmul(pts[c][:], identb[:],
                                             y_acc[:, db, c * TC:(c + 1) * TC],
                                             start=True, stop=False)
                    for j in range(NG):
                        n = g * NG + j
                        a_t = abpool.tile([P, L], BF16, tag="a_t")
                        nc.scalar.activation(a_t[:], dt_q[:], AF.Exp,
                                             scale=A_s[:, db, n:n + 1])
                        b_t = abpool.tile([P, L], BF16, tag="b_t")
                        nc.vector.tensor_tensor(b_t[:], Bg[:, j, :], w_q[:], OP.mult)
                        h_t = hpool.tile([P, L], BF16, tag="h_t")
                        nc.vector.tensor_tensor_scan(
                            h_t[:], a_t[:], b_t[:], 0.0, OP.mult, OP.add)
                        s_t = hpool.tile([P, L], BF16, tag="s_t")
                        # s feeds only PE (a latency-tolerant sink): run ~7/8
                        # of them on the otherwise-idle Pool engine.
                        eng_s = nc.vector if (n % 8 == 3) else nc.gpsimd
                        eng_s.tensor_tensor(s_t[:], h_t[:], Cg[:, j, :], OP.mult)
                        for c in range(NTC):
                            nc.tensor.matmul(pts[c][:], identb[:],
                                             s_t[:, c * TC:(c + 1) * TC],
                                             start=False,
                                             stop=(j == NG - 1))
                    # psum already holds the full partial (seeded): evac
                    if g < NGRP - 1:
                        for c in range(NTC):
                            nc.scalar.copy(
                                y_acc[:, db, c * TC:(c + 1) * TC], pts[c][:])
                    else:
                        sz = zpool.tile([P, L], BF16, tag="szg")
                        nc.sync.dma_start(sz[:], silz_d[:, db, :])
                        for c in range(NTC):
                            cs = slice(c * TC, (c + 1) * TC)
                            nc.vector.scalar_tensor_tensor(
                                y_acc[:, db, cs], pts[c][:], 1.0, sz[:, cs],
                                OP.mult, OP.mult)

        # =============== Phase 7: out_proj + stats + LN ===================
        with ExitStack() as ph:
            pool = ph.enter_context(tc.tile_pool(name="p7", bufs=2))
            pool1 = ph.enter_context(tc.tile_pool(name="p7a", bufs=1))
            ph7s = ExitStack()
            ppsum = ph7s.enter_context(tc.tile_pool(name="ps7", bufs=2, space="PSUM"))
            ppsum1 = ph7s.enter_context(tc.tile_pool(name="ps7p", bufs=1, space="PSUM"))

            wout_s = pool1.tile([P, DB, D], BF16, tag="wout_s")
            nc.sync.dma_start(
                wout_s[:], T["w_out_T"][:].rearrange("(db p) o -> p db o", p=P))
            xs_bf = pool1.tile([P, CB, L], BF16, tag="xs_bf")
            for tcn in range(NTC):
                ts_ = slice(tcn * TC, (tcn + 1) * TC)
                for ob in range(CB):
                    pt = ppsum.tile([P, TC], F32, tag="mmo")
                    for db in range(DB):
                        nc.tensor.matmul(pt[:], wout_s[:, db, ob * P:(ob + 1) * P],
                                         y_acc[:, db, ts_],
                                         start=(db == 0), stop=(db == DB - 1))
                    nc.vector.tensor_copy(xs_bf[:, ob, ts_], pt[:])

            st_both_d = dram.tile([1, 2 * L], F32, tag="st_both_d")
            for tcn in range(NTC):
                ts_ = slice(tcn * TC, (tcn + 1) * TC)
                pt = ppsum.tile([1, TC], F32, tag="red2")
                for cb in range(CB):
                    nc.tensor.matmul(pt[:], ones_bf[:], xs_bf[:, cb, ts_],
                                     start=(cb == 0), stop=(cb == CB - 1))
                prow = small.tile([1, TC], F32, tag="prow")
                nc.vector.tensor_copy(prow[:], pt[:])
                nc.sync.dma_start(st_both_d[:, tcn * TC:(tcn + 1) * TC], prow[:])
                pt2 = ppsum.tile([1, TC], F32, tag="red2")
                for cb in range(CB):
                    sqt = pool.tile([P, TC], BF16, tag="sqt2")
                    nc.vector.tensor_tensor(sqt[:], xs_bf[:, cb, ts_],
                                            xs_bf[:, cb, ts_], OP.mult)
                    nc.tensor.matmul(pt2[:], ones_bf[:], sqt[:],
                                     start=(cb == 0), stop=(cb == CB - 1))
                prow2 = small.tile([1, TC], F32, tag="prow")
                nc.vector.tensor_copy(prow2[:], pt2[:])
                nc.sync.dma_start(st_both_d[:, L + tcn * TC:L + (tcn + 1) * TC],
                                  prow2[:])

            LP2 = 2 * LP

            def permute_both(rowd_in_ap, rowd_out_ap, ppsum1):
                # permute BOTH stat halves of a [1, 2L] row in one pass:
                # tiled form [P, (s f)], partition-permute via perm128, then
                # f-permute within each half via blockdiag perm32.
                s_sb = small.tile([P, LP2], F32, tag="perm_in")
                nc.sync.dma_start(
                    s_sb[:, 0:LP],
                    rowd_in_ap[:, 0:L].rearrange("o (p f) -> (o p) f", p=P))
                nc.sync.dma_start(
                    s_sb[:, LP:LP2],
                    rowd_in_ap[:, L:2 * L].rearrange("o (p f) -> (o p) f", p=P))
                pt = ppsum1.tile([P, LP2], F32, tag="permp")
                nc.tensor.matmul(pt[:], perm128_s[:], s_sb[:], start=True, stop=True)
                u_sb = small.tile([P, LP2], F32, tag="perm_u")
                nc.vector.tensor_copy(u_sb[:], pt[:])
                pt2 = ppsum1.tile([LP2, P], F32, tag="permt")
                nc.tensor.transpose(pt2[:], u_sb[:], ident[:])
                ut = small.tile([LP2, P], F32, tag="perm_ut")
                nc.vector.tensor_copy(ut[:], pt2[:])
                pt3 = ppsum1.tile([LP2, P], F32, tag="permt2")
                nc.tensor.matmul(pt3[:], perm32_s[:], ut[:], start=True, stop=True)
                ut2 = small.tile([LP2, P], F32, tag="perm_ut2")
                nc.vector.tensor_copy(ut2[:], pt3[:])
                pt4 = ppsum1.tile([P, LP2], F32, tag="permp2")
                nc.tensor.transpose(pt4[:], ut2[:], ident[0:LP2, 0:LP2])
                s2_sb = small.tile([P, LP2], F32, tag="perm_out")
                nc.vector.tensor_copy(s2_sb[:], pt4[:])
                if rowd_out_ap is None:
                    return s2_sb
                nc.sync.dma_start(
                    rowd_out_ap[:, 0:L].rearrange("o (p f) -> (o p) f", p=P),
                    s2_sb[:, 0:LP])
                nc.sync.dma_start(
                    rowd_out_ap[:, L:2 * L].rearrange("o (p f) -> (o p) f", p=P),
                    s2_sb[:, LP:LP2])

            permute_both(st_both_d, T["cc_in"], ppsum1)
            ph7s.close()  # free stats/permute PSUM banks for the MLP
            if _NO_COLLECTIVE:
                nc.sync.dma_start(T["cc_out"][:], T["cc_in"][:])
            else:
                nc.gpsimd.collective_compute(
                    "AllReduce", OP.add,
                    replica_groups=[[0, 1], [2, 3], [4, 5], [6, 7]],
                    ins=[T["cc_in"][:]], outs=[T["cc_out"][:]],
                )

            # ---- Phase 9 MLP, emitted here so PE/Act work overlaps the ----
            # ---- AllReduce pair-wait (MLP depends only on xn)          ----
            with ExitStack() as ph9:
                pool9 = ph9.enter_context(tc.tile_pool(name="p9", bufs=2))
                pool91 = ph9.enter_context(tc.tile_pool(name="p9a", bufs=1))
                ppsum9 = ph9.enter_context(tc.tile_pool(name="ps9", bufs=2,
                                                        space="PSUM"))
                w1_s = pool91.tile([P, CB, DI], BF16, tag="w1_s")
                nc.sync.dma_start(
                    w1_s[:], T["w1T"][:].rearrange("(cb p) h -> p cb h", p=P))
                w2_s = pool91.tile([P, DB, D], BF16, tag="w2_s")
                nc.sync.dma_start(
                    w2_s[:], T["w2T"][:].rearrange("(db p) o -> p db o", p=P))
                xn_bf = pool91.tile([P, CB, L], BF16, tag="xn_bf")
                nc.sync.dma_start(xn_bf[:], xn_bf_d[:])
                g_bf = pool91.tile([P, DB, L], BF16, tag="g_bf")
                for tcn in range(NTC):
                    ts_ = slice(tcn * TC, (tcn + 1) * TC)
                    for hb in range(DB):
                        pt = ppsum9.tile([P, TC], F32, tag="mm1")
                        for cb in range(CB):
                            nc.tensor.matmul(
                                pt[:], w1_s[:, cb, hb * P:(hb + 1) * P],
                                xn_bf[:, cb, ts_],
                                start=(cb == 0), stop=(cb == CB - 1))
                        nc.scalar.activation(g_bf[:, hb, ts_], pt[:], AF.Gelu,
                                             bias=b1_s[:, hb:hb + 1])
                    for ob in range(CB):
                        pt = ppsum9.tile([P, TC], F32, tag="mm2")
                        for hb in range(DB):
                            nc.tensor.matmul(
                                pt[:], w2_s[:, hb, ob * P:(ob + 1) * P],
                                g_bf[:, hb, ts_],
                                start=(hb == 0), stop=(hb == DB - 1))
                        ot = pool9.tile([P, TC], F32, tag="oml")
                        nc.vector.tensor_copy(ot[:], pt[:])
                        nc.sync.dma_start(T["out_mlp"][ob * P:(ob + 1) * P, ts_],
                                          ot[:])

            # ---- post-collective LN over the concatenated dirs ----
            ph7t = ExitStack()
            ppsum2 = ph7t.enter_context(tc.tile_pool(name="ps7q", bufs=1,
                                                     space="PSUM"))
            both3 = permute_both(T["cc_out"], None, ppsum2)

            mu3 = both3[:, 0:LP]
            v3 = both3[:, LP:LP2]
            nc.vector.tensor_scalar_mul(mu3, mu3, 1.0 / (2 * D))
            nc.vector.tensor_scalar_mul(v3, v3, 1.0 / (2 * D))
            mu32 = small.tile([P, LP], F32, tag="mu32")
            nc.vector.tensor_tensor(mu32[:], mu3, mu3, OP.mult)
            nc.vector.tensor_sub(v3, v3, mu32[:])
            nc.vector.tensor_scalar_add(v3, v3, 1e-5)
            nc.scalar.sqrt(v3, v3)
            nc.vector.reciprocal(v3, v3)
            nc.vector.tensor_tensor(mu32[:], mu3, v3, OP.mult)
            mr2_rep = pool1.tile([P, L], BF16, tag="mr2_rep")
            rstd2_rep = pool1.tile([P, L], BF16, tag="rstd2_rep")
            replicate_tiled_bf(mu32, mr2_rep, "ln2m")
            replicate_tiled_bf_ap(v3, rstd2_rep, "ln2v")

            xs_ln = pool1.tile([P, CB, L], BF16, tag="xs_ln")
            for cb in range(CB):
                u = pool.tile([P, L], BF16, tag="u_ln2")
                nc.vector.tensor_tensor(u[:], xs_bf[:, cb, :], rstd2_rep[:], OP.mult)
                nc.vector.tensor_sub(u[:], u[:], mr2_rep[:])
                nc.vector.tensor_scalar(xs_ln[:, cb, :], u[:],
                                        lnp_w_s[:, cb:cb + 1],
                                        lnp_b_s[:, cb:cb + 1], OP.mult, OP.add)

            ph7t.close()
            # ---- Phase 8 mixer conv half (PE diag matmuls) ----
            with ExitStack() as ph8:
                pool8 = ph8.enter_context(tc.tile_pool(name="p8", bufs=2))
                pool81 = ph8.enter_context(tc.tile_pool(name="p8a", bufs=1))
                ppsum8 = ph8.enter_context(tc.tile_pool(name="ps8", bufs=4,
                                                        space="PSUM"))
                # padded E/O slab [p, eo, gh, t]; SBUF->SBUF interleave
                # gather: EO[p',eo,gh] = xs_ln channel gh*256 + 2p' + eo
                EO = pool81.tile([P, 2, 2, 2 + L], BF16, tag="EO")
                nc.vector.memset(EO[:, :, :, 0:1], 0.0)
                nc.vector.memset(EO[:, :, :, 1 + L:2 + L], 0.0)
                slab_r = xs_ln[:].rearrange("(a two) cb t -> a two cb t", two=2)
                for eo in range(2):
                    for gh in range(2):
                        for half in range(2):
                            nc.sync.dma_start(
                                EO[half * 64:(half + 1) * 64, eo, gh, 1:1 + L],
                                slab_r[:, eo, gh * 2 + half, :])
                diag_pc = pool81.tile([P, 2, 6, P], BF16, tag="diag_pc")
                for gh in range(2):
                    for i in range(6):
                        nc.vector.tensor_scalar(
                            diag_pc[:, gh, i, :], identb[:], pc_w_s[:, gh, i:i + 1],
                            None, OP.mult)
                for gh in range(2):
                    for tcn in range(NTC):
                        pt = ppsum8.tile([P, TC], F32, tag="mxpsum")
                        for eo in range(2):
                            for k in range(3):
                                nc.tensor.matmul(
                                    pt[:], diag_pc[:, gh, eo * 3 + k, :],
                                    EO[:, eo, gh, k + tcn * TC:k + (tcn + 1) * TC],
                                    start=(eo == 0 and k == 0),
                                    stop=(eo == 1 and k == 2))
                        mout = pool8.tile([P, TC], F32, tag="mout")
                        nc.scalar.activation(mout[:], pt[:], AF.Silu,
                                             bias=pc_b_s[:, gh:gh + 1])
                        nc.sync.dma_start(
                            T["out_mix"][gh * P:(gh + 1) * P,
                                         tcn * TC:(tcn + 1) * TC],
                            mout[:])


_NC_CACHE = None


def _get_nc():
    global _NC_CACHE
    if _NC_CACHE is None:
        _NC_CACHE = _build()
    return _NC_CACHE


def _prep_core_inputs(inputs, b, rev):
    import ml_dtypes
    f32 = np.float32
    bf16 = ml_dtypes.bfloat16

    def dpart(v, nb):  # [nb*128, ...] -> [128, nb, ...]
        v = np.asarray(v, dtype=f32)
        return np.ascontiguousarray(
            v.reshape(nb, P, *v.shape[1:]).transpose(1, 0, *range(2, v.ndim + 1)))

    x = inputs["x"][b]
    if rev:
        x = x[::-1]
    lc_w = inputs["lc_w"][:, 0, :]
    if rev:
        lc_w = lc_w[:, ::-1]
    lnp_w = inputs["lnp_w"][rev * D:(rev + 1) * D]
    lnp_b = inputs["lnp_b"][rev * D:(rev + 1) * D]
    pc_w = inputs["pc_w"][rev * (D // 2):(rev + 1) * (D // 2)]
    if rev:
        pc_w = pc_w[:, :, ::-1]
    pc_b = inputs["pc_b"][rev * (D // 2):(rev + 1) * (D // 2)]
    hsl = slice(rev * DI, (rev + 1) * DI)
    w1 = inputs["w1"][hsl]
    b1v = inputs["b1"][hsl]
    w2 = inputs["w2"][:, hsl]
    A = -np.exp(inputs["A_log"].astype(np.float64)).astype(f32)
    eye = np.eye(P, dtype=f32)
    rv = np.ascontiguousarray(np.eye(P, dtype=f32)[::-1])
    e16 = np.eye(LP, dtype=f32)
    r16 = np.ascontiguousarray(e16[::-1])
    # blockdiag: f-reversal applied independently to the two stat halves
    # of the [1, 2L] row (tiled as [P, (s f)] -> transposed [(s f), P])
    e32 = np.eye(2 * LP, dtype=f32)
    r32 = np.zeros((2 * LP, 2 * LP), f32)
    r32[0:LP, 0:LP] = r16
    r32[LP:, LP:] = r16

    return {
        "x_seq": np.ascontiguousarray(x, dtype=f32),
        "w_in_T": np.ascontiguousarray(inputs["in_w"].astype(f32).T.astype(bf16)),
        "lc_w": dpart(lc_w, CB),
        "lc_b": dpart(inputs["lc_b"], CB),
        "norm_w": dpart(inputs["norm_w"], CB),
        "lnc_w": dpart(inputs["lnc_w"], CB),
        "lnc_b": dpart(inputs["lnc_b"], CB),
        "cv_w": dpart(inputs["cv_w"][:, 0, :], DB),
        "cv_b": dpart(inputs["cv_b"], DB),
        "xp_wT": np.ascontiguousarray(inputs["xp_w"].astype(f32).T.astype(bf16)),
        "dtp_wT": np.ascontiguousarray(inputs["dtp_w"].astype(f32).T.astype(bf16)),
        "dtp_b": dpart(inputs["dtp_b"], DB),
        "A_dn": dpart(A, DB),
        "Dp_dn": dpart(inputs["Dp"], DB),
        "w_out_T": np.ascontiguousarray(inputs["out_w"].astype(f32).T.astype(bf16)),
        "lnp_w": dpart(lnp_w, CB),
        "lnp_b": dpart(lnp_b, CB),
        "pc_w": dpart(np.ascontiguousarray(pc_w).reshape(D // 2, 6), 2),
        "pc_b": dpart(pc_b, 2),
        "w1T": np.ascontiguousarray(np.asarray(w1, dtype=f32).T.astype(bf16)),
        "b1": dpart(b1v, DB),
        "w2T": np.ascontiguousarray(np.asarray(w2, dtype=f32).T.astype(bf16)),
        "perm128": rv if rev else eye,
        "perm32": r32 if rev else e32,
    }


LAST_RESULTS = None


def kernel(**inputs):
    global LAST_RESULTS
    inputs = {k: np.asarray(v) for k, v in inputs.items()}
    nc = _get_nc()
    in_maps = [_prep_core_inputs(inputs, c // 2, c % 2) for c in range(8)]
    res = run_bass_kernel_spmd(nc, in_maps, core_ids=list(range(8)))
    LAST_RESULTS = res
    out = np.empty((B, L, D), np.float32)
    b2 = inputs["b2"].astype(np.float32)
    for b in range(B):
        mf = res.results[2 * b]
        mb = res.results[2 * b + 1]
        acc = inputs["x"][b].astype(np.float32) + b2[None, :]
        acc += mf["out_mlp"].T
        acc += mb["out_mlp"][:, ::-1].T
        acc[:, 0:D // 2] += mf["out_mix"].T
        acc[:, D // 2:] += mb["out_mix"][:, ::-1].T
        out[b] = acc
    return out

